# revision 20
# baseline (speedup 1.0000x reference)
"""Trainium2 Bass kernel for EnhancedVisualOdometryModel.

Data-parallel: 8 samples -> 8 NeuronCores, one sample per core.
All conv compute in float32r (TF32-like, 1 cyc/row on PE), fp32 accumulation.
Convs = shifted-view matmuls over padded SBUF feature tiles.
Correlation = per-row GEMM (x1_row^T @ x2_rows band) -> DRAM bounce -> affine
shear-gather DMAs into channel-major cost volume.
"""

import numpy as np

EPS = 1e-5
MD = 6  # max displacement
ND = 2 * MD + 1  # 13
B, H0, W0 = 8, 256, 512

_CACHE = {}


# ----------------------------------------------------------------------------
# host-side weight preparation
# ----------------------------------------------------------------------------
def _np(x):
    return np.asarray(x, dtype=np.float32)


def _bng(g):
    return _np(g) / np.sqrt(np.float32(1.0 + EPS))


def _convT(w, oscale=None):
    """w [O,I,kh,kw] -> lhsT [I, kh*kw, O], optionally scaling output channels."""
    w = _np(w)
    if oscale is not None:
        w = w * oscale[:, None, None, None]
    O, I, kh, kw = w.shape
    return np.ascontiguousarray(w.transpose(1, 2, 3, 0).reshape(I, kh * kw, O))


def _linT(w, oscale=None, iscale=None):
    """w [O,I] -> lhsT [I,O] with optional per-output / per-input scaling."""
    w = _np(w)
    if oscale is not None:
        w = w * oscale[:, None]
    if iscale is not None:
        w = w * iscale[None, :]
    return np.ascontiguousarray(w.T)


# conv1 piece table: (qy, sy, ry0, nry) with dy = 4*sy + ry - 2*qy
# imx partition layout: p = ry*32 + ic*7 + dx (rows 21..31 of each ry block junk)
# piece lhsT rows zero-padded to match; base partition = 32*ry0, K = 32*(nry-1)+21
C1_PIECES = [
    (0, 0, 0, 4),  # base 0,  K 117
    (0, 1, 0, 3),  # base 0,  K 85
    (1, 0, 2, 2),  # base 64, K 53
    (1, 1, 0, 4),  # base 0,  K 117
    (1, 2, 0, 1),  # base 0,  K 21
]
C1_KP = [32 * (n - 1) + 21 for (_, _, _, n) in C1_PIECES]
C1_BASE = [32 * r0 for (_, _, r0, _) in C1_PIECES]
C1_ROWS = sum(C1_KP)  # 393


def _prep_conv1(w):  # w [64,3,7,7] -> [C1_ROWS, 64]
    w = _np(w)
    rows = []
    for pi, (qy, sy, ry0, nry) in enumerate(C1_PIECES):
        blk = np.zeros((C1_KP[pi], 64), np.float32)
        for j in range(nry):
            ry = ry0 + j
            dy = 4 * sy + ry - 2 * qy
            for ic in range(3):
                for dx in range(7):
                    blk[j * 32 + ic * 7 + dx] = w[:, ic, dy, dx]
        rows.append(blk)
    return np.concatenate(rows, axis=0)


def _emb169(idx_o, idx_i, w):
    """embed [168-ch] conv weights into 169-ch (dy*13+dx) layout with center hole."""
    # orig channel order: (i,j) for i in -6..6, j in -6..6, skipping (0,0)
    # -> orig index o: flat = (i+6)*13 + (j+6); o = flat - (flat > 84)
    return None


def _o169_map():
    m = np.zeros(169, np.int64)
    valid = np.ones(169, bool)
    k = 0
    for f in range(169):
        if f == 84:
            valid[f] = False
            m[f] = 0
        else:
            m[f] = k
            k += 1
    return m, valid


def _prep_weights(p):
    m169, v169 = _o169_map()
    W = {}

    W["w_c1"] = _prep_conv1(p["conv1_w"])
    W["c1_g"] = _bng(p["bn1_g"])
    W["c1_b"] = _np(p["bn1_b"])

    for li, layer in enumerate(["layer1", "layer2", "layer3"]):
        for bi, blk in enumerate(p[layer]):
            pre = f"l{li + 1}{bi}"
            g1, b1 = _bng(blk["g1"]), _np(blk["b1"])
            g2, b2 = _bng(blk["g2"]), _np(blk["b2"])
            W[f"{pre}_w1"] = _convT(blk["c1"])
            W[f"{pre}_g1"] = g1
            W[f"{pre}_b1"] = b1
            W[f"{pre}_w2"] = _convT(blk["c2"], oscale=g2)  # g2 folded
            W[f"{pre}_b2"] = b2
            if "cd" in blk:
                gd, bd = _bng(blk["gd"]), _np(blk["bd"])
                W[f"{pre}_wd"] = _linT(_np(blk["cd"])[:, :, 0, 0], oscale=gd)
                W[f"{pre}_bd"] = bd

    g = _bng(p["ref1_g"])
    W["ref1_w"] = _convT(p["ref1_w"], oscale=g)
    W["ref1_b"] = _np(p["ref1_b"]) * g + _np(p["ref1_bb"])
    g = _bng(p["ref2_g"])
    W["ref2_w"] = _convT(p["ref2_w"], oscale=g)
    W["ref2_b"] = _np(p["ref2_b"]) * g + _np(p["ref2_bb"])

    W["gc1_w"] = _linT(_np(p["gc1_w"])[:, :, 0, 0], iscale=np.full(256, 1.0 / 512, np.float32))
    W["gc1_b"] = _np(p["gc1_b"])
    W["gc2_w"] = _linT(_np(p["gc2_w"])[:, :, 0, 0])
    W["gc2_b"] = _np(p["gc2_b"])

    # attention 1x1: embed 168 -> 169
    aw = _np(p["att_w"])[:, :, 0, 0]  # [168, 168]
    a169 = np.zeros((169, 169), np.float32)
    io = np.where(v169)[0]
    a169[np.ix_(io, io)] = aw[np.ix_(m169[io], m169[io])].T  # lhsT [in169, out169]
    W["att_w"] = a169
    ab = np.zeros(169, np.float32)
    ab[io] = _np(p["att_b"])[m169[io]]
    W["att_b"] = ab

    # cc1: [256, 168, 3, 3] -> lhsT [169, 9, 256]
    g = _bng(p["cc1_g"])
    c1t = _convT(p["cc1_w"], oscale=g)  # [168, 9, 256]
    cc1 = np.zeros((169, 9, 256), np.float32)
    cc1[io] = c1t[m169[io]]
    W["cc1_w"] = cc1
    W["cc1_b"] = _np(p["cc1_b"]) * g + _np(p["cc1_bb"])
    g = _bng(p["cc2_g"])
    W["cc2_w"] = _convT(p["cc2_w"], oscale=g)
    W["cc2_b"] = _np(p["cc2_b"]) * g + _np(p["cc2_bb"])
    g = _bng(p["cc3_g"])
    W["cc3_w"] = _convT(p["cc3_w"], oscale=g)
    W["cc3_b"] = _np(p["cc3_b"]) * g + _np(p["cc3_bb"])

    g = _bng(p["sb1_g"])
    W["sb1_w"] = _linT(p["sb1_w"], oscale=g, iscale=np.full(256, 1.0 / 512, np.float32))
    W["sb1_b"] = _np(p["sb1_b"]) * g + _np(p["sb1_bb"])
    g = _bng(p["sb2_g"])
    W["sb2_w"] = _linT(p["sb2_w"], oscale=g)
    W["sb2_b"] = _np(p["sb2_b"]) * g + _np(p["sb2_bb"])
    W["sb3_w"] = _linT(p["sb3_w"])
    W["sb3_b"] = _np(p["sb3_b"])

    isc = np.concatenate(
        [
            np.full(64, 1.0 / 2048, np.float32),
            np.full(128, 1.0 / 512, np.float32),
            np.full(128, 1.0 / 512, np.float32),
            np.ones(128, np.float32),
        ]
    )
    g = _bng(p["fc1_g"])
    W["fc1_w"] = _linT(p["fc1_w"], oscale=g, iscale=isc)
    W["fc1_b"] = _np(p["fc1_b"]) * g + _np(p["fc1_bb"])
    g = _bng(p["fc2_g"])
    W["fc2_w"] = _linT(p["fc2_w"], oscale=g)
    W["fc2_b"] = _np(p["fc2_b"]) * g + _np(p["fc2_bb"])
    W["fc3_w"] = _linT(p["fc3_w"])
    W["fc3_b"] = _np(p["fc3_b"])
    return W


# ----------------------------------------------------------------------------
# device program
# ----------------------------------------------------------------------------
def _build():
    import concourse.bass as bass
    import concourse.mybir as mybir
    import concourse.tile as tile
    from concourse import bacc
    from contextlib import ExitStack

    f32 = mybir.dt.float32
    f32r = mybir.dt.float32r
    AF = mybir.ActivationFunctionType
    ALU = mybir.AluOpType
    AX = mybir.AxisListType

    nc = bacc.Bacc(None)

    # ---- parameters -------------------------------------------------------
    def par(name, shape):
        return nc.declare_dram_parameter(name, list(shape), f32, isOutput=False)

    P = {}
    P["img1"] = par("img1", (3, 256, 512))
    P["img2"] = par("img2", (3, 256, 512))
    P["w_c1"] = par("w_c1", (C1_ROWS, 64))
    P["c1_g"] = par("c1_g", (64,))
    P["c1_b"] = par("c1_b", (64,))
    for pre, ic, oc, down in [
        ("l10", 64, 64, False),
        ("l11", 64, 64, False),
        ("l20", 64, 128, True),
        ("l21", 128, 128, False),
        ("l30", 128, 256, True),
        ("l31", 256, 256, False),
    ]:
        P[f"{pre}_w1"] = par(f"{pre}_w1", (ic, 9, oc))
        P[f"{pre}_g1"] = par(f"{pre}_g1", (oc,))
        P[f"{pre}_b1"] = par(f"{pre}_b1", (oc,))
        P[f"{pre}_w2"] = par(f"{pre}_w2", (oc, 9, oc))
        P[f"{pre}_b2"] = par(f"{pre}_b2", (oc,))
        if down:
            P[f"{pre}_wd"] = par(f"{pre}_wd", (ic, oc))
            P[f"{pre}_bd"] = par(f"{pre}_bd", (oc,))
    P["ref1_w"] = par("ref1_w", (256, 9, 256))
    P["ref1_b"] = par("ref1_b", (256,))
    P["ref2_w"] = par("ref2_w", (256, 9, 128))
    P["ref2_b"] = par("ref2_b", (128,))
    P["gc1_w"] = par("gc1_w", (256, 128))
    P["gc1_b"] = par("gc1_b", (128,))
    P["gc2_w"] = par("gc2_w", (128, 128))
    P["gc2_b"] = par("gc2_b", (128,))
    P["att_w"] = par("att_w", (169, 169))
    P["att_b"] = par("att_b", (169,))
    P["cc1_w"] = par("cc1_w", (169, 9, 256))
    P["cc1_b"] = par("cc1_b", (256,))
    P["cc2_w"] = par("cc2_w", (256, 9, 128))
    P["cc2_b"] = par("cc2_b", (128,))
    P["cc3_w"] = par("cc3_w", (128, 9, 64))
    P["cc3_b"] = par("cc3_b", (64,))
    P["sb1_w"] = par("sb1_w", (256, 128))
    P["sb1_b"] = par("sb1_b", (128,))
    P["sb2_w"] = par("sb2_w", (128, 64))
    P["sb2_b"] = par("sb2_b", (64,))
    P["sb3_w"] = par("sb3_w", (64, 1))
    P["sb3_b"] = par("sb3_b", (1,))
    P["fc1_w"] = par("fc1_w", (448, 512))
    P["fc1_b"] = par("fc1_b", (512,))
    P["fc2_w"] = par("fc2_w", (512, 256))
    P["fc2_b"] = par("fc2_b", (256,))
    P["fc3_w"] = par("fc3_w", (256, 3))
    P["fc3_b"] = par("fc3_b", (3,))

    out_trans = nc.declare_dram_parameter("trans", [3], f32, isOutput=True)

    ctx = ExitStack()
    tc = ctx.enter_context(tile.TileContext(nc))

    vecs = ctx.enter_context(tc.tile_pool(name="vecs", bufs=1))
    wpool = ctx.enter_context(tc.tile_pool(name="wpool", bufs=2))
    psum = ctx.enter_context(tc.tile_pool(name="psum", bufs=4, space="PSUM"))
    psumR = ctx.enter_context(tc.tile_pool(name="psumR", bufs=2, space="PSUM"))
    dram = ctx.enter_context(tc.tile_pool(name="dram", bufs=1, space="DRAM"))

    # ---- small helpers ----------------------------------------------------
    def vec_tile(name, param, n, dt=f32):
        t = vecs.tile([n, 1], dt, name=name)
        src = param[:]
        ap = bass.AP(tensor=src.tensor, offset=src.offset, ap=[[1, n], [0, 1]])
        nc.sync.dma_start(out=t, in_=ap.bitcast(dt))
        return t

    BIAS = {}

    def bias_of(pname, n, dt=f32):
        key = (pname, n, dt)
        if key not in BIAS:
            BIAS[key] = vec_tile(f"bv_{pname}_{n}", P[pname], n, dt)
        return BIAS[key]

    def load_w(pname, shape, name=None, pool=None):
        """DMA a weight param (or slice) into an f32r SBUF tile."""
        pl = pool or wpool
        t = pl.tile(list(shape), f32r, name=name or "w")
        ap = P[pname][:]
        nc.sync.dma_start(out=t, in_=ap.bitcast(f32r))
        return t

    def act(out_ap, in_ap, func, bias=0.0, scale=1.0):
        nc.scalar.activation(out_ap, in_ap, func, bias=bias, scale=scale)

    # generic 3x3 conv over padded tiles --------------------------------
    # src_blocks: list of (tile, oy, ox) each holding <=128 channels
    # kcs: channel count per src block; w param name with shape [ICtot, 9, OCtot]
    def conv3(
        name,
        src_blocks,
        kcs,
        wname,
        octot,
        H,
        W,
        CR,
        post,
        stride=1,
    ):
        ictot = sum(kcs)
        wts = []
        k0 = 0
        for i, kc in enumerate(kcs):
            t = wpool.tile([128, 9, octot], f32r, name="w")
            ap = P[wname][k0 : k0 + kc, :, :]
            nc.sync.dma_start(out=t[:kc], in_=ap.bitcast(f32r))
            wts.append(t)
            k0 += kc
        mlist = [(m0, min(128, octot - m0)) for m0 in range(0, octot, 128)]
        for mi, (m0, mb) in enumerate(mlist):
            for y0 in range(0, H, CR):
                rows = min(CR, H - y0)
                p = psum.tile([128, CR, W], mybir.dt.float32, name="cp")
                first = True
                nmm = len(kcs) * 9
                k = 0
                for ki, kc in enumerate(kcs):
                    st, soy, sox = src_blocks[ki]
                    for t in range(9):
                        ty, tx = divmod(t, 3)
                        r0 = soy + stride * y0 + ty - 1
                        c0 = sox + tx - 1
                        rhs = st[
                            :kc,
                            r0 : r0 + stride * rows : stride,
                            c0 : c0 + stride * W : stride,
                        ]
                        k += 1
                        nc.tensor.matmul(
                            p[:mb, :rows, :],
                            wts[ki][:kc, t, m0 : m0 + mb],
                            rhs,
                            start=first,
                            stop=(k == nmm),
                        )
                        first = False
                post(p, mi, m0, mb, y0, rows)

    def post_act(dsts, func, gtile=None, btile=None, W=None):
        """dsts: list of (tile, oy, ox) per M-chunk."""

        def post(p, mi, m0, mb, y0, rows):
            dt_, oy, ox = dsts[mi]
            sc = gtile[mi] if isinstance(gtile, list) else (gtile if gtile is not None else 1.0)
            bi = btile[mi] if isinstance(btile, list) else (btile if btile is not None else 0.0)
            act(
                dt_[:mb, oy + y0 : oy + y0 + rows, ox : ox + W],
                p[:mb, :rows, :],
                func,
                bias=bi,
                scale=sc,
            )

        return post

    def post_res(dsts, idns, btile, W):
        """residual: dst = relu(psum + idn + b). idns: (tile, oy, ox) per chunk."""

        def post(p, mi, m0, mb, y0, rows):
            dt_, oy, ox = dsts[mi]
            it_, ioy, iox = idns[mi]
            dv = dt_[:mb, oy + y0 : oy + y0 + rows, ox : ox + W]
            nc.vector.tensor_tensor(
                out=dv,
                in0=p[:mb, :rows, :],
                in1=it_[:mb, ioy + y0 : ioy + y0 + rows, iox : iox + W],
                op=ALU.add,
            )
            b = btile[mi] if isinstance(btile, list) else btile
            act(dv, dv.bitcast(f32), AF.Relu, bias=b)

        return post

    def memset_border(t, C, Hp, Wp, oy, ox, H, W, val=0.0):
        for ap in [
            t[:C, 0:oy, :] if oy > 0 else None,
            t[:C, oy + H : Hp, :] if oy + H < Hp else None,
            t[:C, :, 0:ox] if ox > 0 else None,
            t[:C, :, ox + W : Wp] if ox + W < Wp else None,
        ]:
            if ap is not None:
                nc.vector.memset(ap.bitcast(f32), val)

    def conv1x1_s2(src, soy, sox, kc, wt, octot, H, W, CR, post):
        mlist = [(m0, min(128, octot - m0)) for m0 in range(0, octot, 128)]
        for mi, (m0, mb) in enumerate(mlist):
            for y0 in range(0, H, CR):
                rows = min(CR, H - y0)
                p = psum.tile([128, CR, W], mybir.dt.float32, name="cp")
                rhs = src[
                    :kc,
                    soy + 2 * y0 : soy + 2 * y0 + 2 * rows : 2,
                    sox : sox + 2 * W : 2,
                ]
                nc.tensor.matmul(
                    p[:mb, :rows, :], wt[:kc, m0 : m0 + mb], rhs, start=True, stop=True
                )
                post(p, mi, m0, mb, y0, rows)

    # residual basic block (stride 1), in-place: io tile holds in & out
    def basic_block(pre, io, C, H, W, mpool, mtag):
        g1 = bias_of(f"{pre}_g1", C)
        b1 = bias_of(f"{pre}_b1", C)
        b2 = bias_of(f"{pre}_b2", C)
        m = mpool.tile([C, H + 2, W + 2], f32r, name=mtag)
        memset_border(m, C, H + 2, W + 2, 1, 1, H, W)
        CR = max(1, 512 // W)
        conv3(
            f"{pre}c1",
            [(io, 1, 1)],
            [C],
            f"{pre}_w1",
            C,
            H,
            W,
            CR,
            post_act([(m, 1, 1)], AF.Relu, gtile=g1, btile=b1, W=W),
        )
        conv3(
            f"{pre}c2",
            [(m, 1, 1)],
            [C],
            f"{pre}_w2",
            C,
            H,
            W,
            CR,
            post_res([(io, 1, 1)], [(io, 1, 1)], b2, W),
        )

    def zero_dram(zt, dt_, nelem):
        # dt_ flat [nelem], nelem divisible by 2145; zeros in [128,2145] chunks
        step = 128 * 2145
        off = 0
        while off < nelem:
            n = min(step, nelem - off)
            rows = n // 2145
            dst = bass.AP(
                tensor=dt_.tensor,
                offset=dt_.offset + off,
                ap=[[2145, rows], [1, 2145]],
            )
            nc.sync.dma_start(out=dst, in_=zt[:rows, :])
            off += rows * 2145

    # ---- per-image feature extraction -------------------------------------
    pool1pool = ctx.enter_context(tc.tile_pool(name="pool1pool", bufs=1))
    lowpads = ctx.enter_context(tc.tile_pool(name="lowpads", bufs=2))
    feats = {}

    EV_H, EV_W = 264, 260

    for ii, iname in enumerate(["img1", "img2"]):
        img = P[iname]
        with ExitStack() as imctx:
            # --- phase A: conv1 + maxpool ---
            actx = imctx.enter_context(ExitStack())
            prectx = ExitStack()
            mpoolz = prectx.enter_context(tc.tile_pool(name=f"zt_{ii}", bufs=1))
            zt = mpoolz.tile([128, 2145], f32, name="zt")
            nc.vector.memset(zt, 0.0)
            evb = dram.tile([3 * EV_H * EV_W], f32, name=f"evb{ii}")
            odb = dram.tile([3 * EV_H * EV_W], f32, name=f"odb{ii}")
            zero_dram(zt, evb, 3 * EV_H * EV_W)
            zero_dram(zt, odb, 3 * EV_H * EV_W)
            evs = [
                bass.AP(tensor=evb.tensor, offset=evb.offset + ic * EV_H * EV_W, ap=[[1, 1]])
                for ic in range(3)
            ]
            ods = [
                bass.AP(tensor=odb.tensor, offset=odb.offset + ic * EV_H * EV_W, ap=[[1, 1]])
                for ic in range(3)
            ]

            # load image rows onto partitions: I[p, n, w] = img[row p*8+n]
            it = mpoolz.tile([128, 8, 512], f32, name="imgrows")
            iap = img[:, :, :]
            srcv = bass.AP(
                tensor=iap.tensor,
                offset=iap.offset,
                ap=[[8 * 512, 96], [512, 8], [1, 512]],
            )
            nc.sync.dma_start(out=it[:96], in_=srcv)
            # split phases on DVE
            et = mpoolz.tile([128, 8, 256], f32, name="evsplit")  # img even cols
            ot = mpoolz.tile([128, 8, 256], f32, name="odsplit")  # img odd cols
            nc.vector.tensor_copy(et[:96], it[:96, :, 0:512:2])
            nc.vector.tensor_copy(ot[:96], it[:96, :, 1:512:2])
            # even_img[r, u] = img[r, 2u-3] (odd cols, u in [2,257])
            # odd_img[r, u]  = img[r, 2u-2] (even cols, u in [1,256])
            for ic in range(3):
                dst = bass.AP(
                    tensor=evb.tensor,
                    offset=evb.offset + ic * EV_H * EV_W + 3 * EV_W + 2,
                    ap=[[8 * EV_W, 32], [EV_W, 8], [1, 256]],
                )
                nc.sync.dma_start(out=dst, in_=ot[32 * ic : 32 * ic + 32])
                dst = bass.AP(
                    tensor=odb.tensor,
                    offset=odb.offset + ic * EV_H * EV_W + 3 * EV_W + 1,
                    ap=[[8 * EV_W, 32], [EV_W, 8], [1, 256]],
                )
                nc.sync.dma_start(out=dst, in_=et[32 * ic : 32 * ic + 32])

            prectx.close()
            pool1 = pool1pool.tile([64, 66, 130], f32r, name="pool1")
            memset_border(pool1, 64, 66, 130, 1, 1, 64, 128)

            c1g = bias_of("c1_g", 64)
            c1b = bias_of("c1_b", 64)
            wc1pool = actx.enter_context(tc.tile_pool(name=f"wc1_{ii}", bufs=5))
            wc1 = []
            r0 = 0
            for pi in range(5):
                K = C1_KP[pi]
                bse = C1_BASE[pi]
                t = wc1pool.tile([117, 64], f32r, name="wc1p")
                nc.sync.dma_start(
                    out=t[bse : bse + K], in_=P["w_c1"][r0 : r0 + K, :].bitcast(f32r)
                )
                wc1.append((t, K))
                r0 += K

            impool = actx.enter_context(tc.tile_pool(name=f"imx_{ii}", bufs=1))
            slabpool = actx.enter_context(tc.tile_pool(name=f"c1s_{ii}", bufs=2))
            tvpool = actx.enter_context(tc.tile_pool(name=f"tv_{ii}", bufs=1))

            for s in range(4):
                # imx slab rows Y'' in [Ib, Ib+nrowsY)
                Ib = max(0, 16 * s - 1)
                Ie = min(66, 16 * s + 19)
                nY = Ie - Ib
                imx = impool.tile([128, 20, 256], f32r, name="imx")
                for ry in range(4):
                    for ic in range(3):
                        pbase = ry * 32 + ic * 7
                        # even dx (0,2,4,6): u = x + dx/2 from even plane
                        src = bass.AP(
                            tensor=evs[ic].tensor,
                            offset=evs[ic].offset + (4 * Ib + ry) * EV_W + 0,
                            ap=[[1, 4], [4 * EV_W, nY], [1, 256]],
                        )
                        nc.sync.dma_start(
                            out=imx[pbase : pbase + 7 : 2, :nY, :],
                            in_=src.bitcast(f32r),
                        )
                        # odd dx (1,3,5): u = x + (dx-1)/2 from odd plane
                        src = bass.AP(
                            tensor=ods[ic].tensor,
                            offset=ods[ic].offset + (4 * Ib + ry) * EV_W + 0,
                            ap=[[1, 3], [4 * EV_W, nY], [1, 256]],
                        )
                        nc.sync.dma_start(
                            out=imx[pbase + 1 : pbase + 6 : 2, :nY, :],
                            in_=src.bitcast(f32r),
                        )

                # c1 slab: c1pad rows [32s, 32s+34) ; c1pad row r = c1 row r-1
                slab = slabpool.tile([64, 34, 258], f32, name="c1s")
                nc.vector.memset(slab[:, :, 0:1], -1e30)
                nc.vector.memset(slab[:, :, 257:258], -1e30)
                if s == 0:
                    nc.vector.memset(slab[:, 0:1, :], -1e30)
                if s == 3:
                    nc.vector.memset(slab[:, 33:34, :], -1e30)
                rlo = max(0, 32 * s - 1)
                rhi = min(128, 32 * s + 33)
                for qy in (0, 1):
                    Ys = [Y for Y in range(64) if rlo <= 2 * Y + qy < rhi]
                    pieces = [pc for pc in range(5) if C1_PIECES[pc][0] == qy]
                    i = 0
                    while i < len(Ys):
                        # chunk of up to 2 consecutive Y
                        cn = 1
                        if i + 1 < len(Ys) and Ys[i + 1] == Ys[i] + 1:
                            cn = 2
                        Y = Ys[i]
                        p = psum.tile([64, 2, 256], mybir.dt.float32, name="cp")
                        for j, pc in enumerate(pieces):
                            _, sy, ry0, nry = C1_PIECES[pc]
                            K = C1_KP[pc]
                            bse = C1_BASE[pc]
                            wt, _ = wc1[pc]
                            rhs = imx[
                                bse : bse + K, Y + sy - Ib : Y + sy - Ib + cn, :
                            ]
                            nc.tensor.matmul(
                                p[:, :cn, :],
                                wt[C1_BASE[pc] : C1_BASE[pc] + K],
                                rhs,
                                start=(j == 0),
                                stop=(j == len(pieces) - 1),
                            )
                        r = 2 * Y + qy  # first c1 row of chunk; rows r, r+2
                        si = r + 1 - 32 * s  # slab row index
                        act(
                            slab[:, si : si + 2 * cn - 1 : 2, 1:257],
                            p[:, :cn, :],
                            AF.Relu,
                            bias=c1b,
                            scale=c1g,
                        )
                        i += cn

                # maxpool slab -> pool1 rows [1+16s, 1+16s+16)
                tv = tvpool.tile([64, 16, 258], f32, name="tv")
                nc.vector.tensor_tensor(
                    out=tv, in0=slab[:, 0:32:2, :], in1=slab[:, 1:33:2, :], op=ALU.max
                )
                nc.vector.tensor_tensor(
                    out=tv, in0=tv, in1=slab[:, 2:34:2, :], op=ALU.max
                )
                q0 = 1 + 16 * s
                dstv = pool1[:, q0 : q0 + 16, 1:129]
                nc.vector.tensor_tensor(
                    out=dstv, in0=tv[:, :, 0:256:2], in1=tv[:, :, 1:257:2], op=ALU.max
                )
                nc.vector.tensor_tensor(
                    out=dstv, in0=dstv.bitcast(f32), in1=tv[:, :, 2:258:2], op=ALU.max
                )

            actx.close()

            # --- phase B: layer1 (in-place on pool1) ---
            with ExitStack() as bctx:
                mpool = bctx.enter_context(tc.tile_pool(name=f"l1m_{ii}", bufs=1))
                basic_block("l10", pool1, 64, 64, 128, mpool, "l1m")
                basic_block("l11", pool1, 64, 64, 128, mpool, "l1m")

                # --- phase C: layer2 ---
                l2pool = bctx.enter_context(tc.tile_pool(name=f"l2_{ii}", bufs=1))
                m2pool = bctx.enter_context(tc.tile_pool(name=f"l2m_{ii}", bufs=2))
                # l20: conv1 s2 64->128
                g1 = bias_of("l20_g1", 128)
                b1 = bias_of("l20_b1", 128)
                m2 = m2pool.tile([128, 34, 66], f32r, name="l2m")
                memset_border(m2, 128, 34, 66, 1, 1, 32, 64)
                conv3(
                    "l20c1",
                    [(pool1, 1, 1)],
                    [64],
                    "l20_w1",
                    128,
                    32,
                    64,
                    8,
                    post_act([(m2, 1, 1)], AF.Relu, gtile=g1, btile=b1, W=64),
                    stride=2,
                )
                # downsample idn
                wd = load_w("l20_wd", (64, 128))
                bd = bias_of("l20_bd", 128)
                idn2 = l2pool.tile([128, 32, 64], f32, name="l2idn")
                conv1x1_s2(
                    pool1,
                    1,
                    1,
                    64,
                    wd,
                    128,
                    32,
                    64,
                    8,
                    post_act([(idn2, 0, 0)], AF.Identity, btile=bd, W=64),
                )
                b2 = bias_of("l20_b2", 128)
                io2 = l2pool.tile([128, 34, 66], f32r, name="l2io")
                memset_border(io2, 128, 34, 66, 1, 1, 32, 64)
                conv3(
                    "l20c2",
                    [(m2, 1, 1)],
                    [128],
                    "l20_w2",
                    128,
                    32,
                    64,
                    8,
                    post_res([(io2, 1, 1)], [(idn2, 0, 0)], b2, 64),
                )
                # l21 in-place on io2, but output goes to lowpad
                g1 = bias_of("l21_g1", 128)
                b1 = bias_of("l21_b1", 128)
                b2 = bias_of("l21_b2", 128)
                m2b = m2pool.tile([128, 34, 66], f32r, name="l2m")
                memset_border(m2b, 128, 34, 66, 1, 1, 32, 64)
                conv3(
                    "l21c1",
                    [(io2, 1, 1)],
                    [128],
                    "l21_w1",
                    128,
                    32,
                    64,
                    8,
                    post_act([(m2b, 1, 1)], AF.Relu, gtile=g1, btile=b1, W=64),
                )
                low = lowpads.tile([128, 44, 76], f32r, name="low")
                memset_border(low, 128, 44, 76, 6, 6, 32, 64)
                conv3(
                    "l21c2",
                    [(m2b, 1, 1)],
                    [128],
                    "l21_w2",
                    128,
                    32,
                    64,
                    8,
                    post_res([(low, 6, 6)], [(io2, 1, 1)], b2, 64),
                )

                # --- phase D: layer3 ---
                l3pool = bctx.enter_context(tc.tile_pool(name=f"l3_{ii}", bufs=1))
                # l30 conv1 s2 128->256
                def bias_pair(pname, n):
                    key = (pname, "pair")
                    if key not in BIAS:
                        t1 = vecs.tile([128, 1], f32, name=f"bv_{pname}_0")
                        t2 = vecs.tile([128, 1], f32, name=f"bv_{pname}_1")
                        src = P[pname][:]
                        ap1 = bass.AP(tensor=src.tensor, offset=src.offset, ap=[[1, 128], [0, 1]])
                        ap2 = bass.AP(tensor=src.tensor, offset=src.offset + 128, ap=[[1, 128], [0, 1]])
                        nc.sync.dma_start(out=t1, in_=ap1)
                        nc.sync.dma_start(out=t2, in_=ap2)
                        BIAS[key] = [t1, t2]
                    return BIAS[key]

                g1p = bias_pair("l30_g1", 256)
                b1p = bias_pair("l30_b1", 256)
                m3a = l3pool.tile([128, 18, 34], f32r, name="l3ma")
                m3b = l3pool.tile([128, 18, 34], f32r, name="l3mb")
                for t in (m3a, m3b):
                    memset_border(t, 128, 18, 34, 1, 1, 16, 32)
                conv3(
                    "l30c1",
                    [(low, 6, 6)],
                    [128],
                    "l30_w1",
                    256,
                    16,
                    32,
                    16,
                    post_act([(m3a, 1, 1), (m3b, 1, 1)], AF.Relu, gtile=g1p, btile=b1p, W=32),
                    stride=2,
                )
                wd = load_w("l30_wd", (128, 256))
                bdp = bias_pair("l30_bd", 256)
                idn3a = l3pool.tile([128, 16, 32], f32, name="l3ia")
                idn3b = l3pool.tile([128, 16, 32], f32, name="l3ib")
                conv1x1_s2(
                    low,
                    6,
                    6,
                    128,
                    wd,
                    256,
                    16,
                    32,
                    16,
                    post_act([(idn3a, 0, 0), (idn3b, 0, 0)], AF.Identity, btile=bdp, W=32),
                )
                b2p = bias_pair("l30_b2", 256)
                h_a = l3pool.tile([128, 18, 34], f32r, name="h_a")
                h_b = l3pool.tile([128, 18, 34], f32r, name="h_b")
                for t in (h_a, h_b):
                    memset_border(t, 128, 18, 34, 1, 1, 16, 32)
                conv3(
                    "l30c2",
                    [(m3a, 1, 1), (m3b, 1, 1)],
                    [128, 128],
                    "l30_w2",
                    256,
                    16,
                    32,
                    16,
                    post_res(
                        [(h_a, 1, 1), (h_b, 1, 1)],
                        [(idn3a, 0, 0), (idn3b, 0, 0)],
                        b2p,
                        32,
                    ),
                )
                # l31 (256ch, in-place on h_a/h_b)
                g1p = bias_pair("l31_g1", 256)
                b1p = bias_pair("l31_b1", 256)
                b2p = bias_pair("l31_b2", 256)
                m3c = l3pool.tile([128, 18, 34], f32r, name="l3mc")
                m3d = l3pool.tile([128, 18, 34], f32r, name="l3md")
                for t in (m3c, m3d):
                    memset_border(t, 128, 18, 34, 1, 1, 16, 32)
                conv3(
                    "l31c1",
                    [(h_a, 1, 1), (h_b, 1, 1)],
                    [128, 128],
                    "l31_w1",
                    256,
                    16,
                    32,
                    16,
                    post_act([(m3c, 1, 1), (m3d, 1, 1)], AF.Relu, gtile=g1p, btile=b1p, W=32),
                )
                conv3(
                    "l31c2",
                    [(m3c, 1, 1), (m3d, 1, 1)],
                    [128, 128],
                    "l31_w2",
                    256,
                    16,
                    32,
                    16,
                    post_res(
                        [(h_a, 1, 1), (h_b, 1, 1)],
                        [(h_a, 1, 1), (h_b, 1, 1)],
                        b2p,
                        32,
                    ),
                )

                # gc pooling (sum over spatial of high)
                gsa = vecs.tile([128, 1], f32, name=f"gsa{ii}")
                gsb = vecs.tile([128, 1], f32, name=f"gsb{ii}")
                with nc.allow_low_precision(reason="f32r out, fp32 bits"):
                    nc.vector.tensor_reduce(gsa, h_a[:, 1:17, 1:33], AX.XY, ALU.add)
                    nc.vector.tensor_reduce(gsb, h_b[:, 1:17, 1:33], AX.XY, ALU.add)

                # --- phase E: refinement convs ---
                r1p = bias_pair("ref1_b", 256)
                ra = l3pool.tile([128, 18, 34], f32r, name="ra")
                rb = l3pool.tile([128, 18, 34], f32r, name="rb")
                for t in (ra, rb):
                    memset_border(t, 128, 18, 34, 1, 1, 16, 32)
                conv3(
                    "ref1",
                    [(h_a, 1, 1), (h_b, 1, 1)],
                    [128, 128],
                    "ref1_w",
                    256,
                    16,
                    32,
                    16,
                    post_act([(ra, 1, 1), (rb, 1, 1)], AF.Relu, btile=r1p, W=32),
                )
                r2b = bias_of("ref2_b", 128)
                rr = l3pool.tile([128, 16, 32], f32, name="rr")
                conv3(
                    "ref2",
                    [(ra, 1, 1), (rb, 1, 1)],
                    [128, 128],
                    "ref2_w",
                    128,
                    16,
                    32,
                    16,
                    post_act([(rr, 0, 0)], AF.Relu, btile=r2b, W=32),
                )
                rsum = vecs.tile([128, 1], f32, name=f"rsum{ii}")
                with nc.allow_low_precision(reason="f32r out, fp32 bits"):
                    nc.vector.tensor_reduce(rsum, rr, AX.XY, ALU.add)

                # gc head: gc1 (relu) -> gc2 (sigmoid)
                wg1a = wpool.tile([128, 128], f32, name="w")
                nc.sync.dma_start(out=wg1a, in_=P["gc1_w"][0:128, :])
                wg1b = wpool.tile([128, 128], f32, name="w")
                nc.sync.dma_start(out=wg1b, in_=P["gc1_w"][128:256, :])
                gb1 = bias_of("gc1_b", 128)
                pg = psum.tile([128, 2, 256], mybir.dt.float32, name="cp")
                nc.tensor.matmul(pg[:, 0, 0:1], wg1a, gsa, start=True, stop=False)
                nc.tensor.matmul(pg[:, 0, 0:1], wg1b, gsb, start=False, stop=True)
                gv = vecs.tile([128, 1], f32, name=f"gv{ii}")
                act(gv, pg[:, 0, 0:1], AF.Relu, bias=gb1)
                wg2 = wpool.tile([128, 128], f32, name="w")
                nc.sync.dma_start(out=wg2, in_=P["gc2_w"][:, :])
                gb2 = bias_of("gc2_b", 128)
                pg2 = psum.tile([128, 2, 256], mybir.dt.float32, name="cp")
                nc.tensor.matmul(pg2[:, 0, 0:1], wg2, gv, start=True, stop=True)
                gcv = vecs.tile([128, 1], f32, name=f"gcv{ii}")
                act(gcv, pg2[:, 0, 0:1], AF.Sigmoid, bias=gb2)

            feats[ii] = dict(low=low, rsum=rsum, gcv=gcv)

    # ---- correlation ------------------------------------------------------
    low1 = feats[0]["low"]
    low2 = feats[1]["low"]
    inv_sqrt_c = float(1.0 / np.sqrt(128.0))

    with ExitStack() as cctx:
        rpool = cctx.enter_context(tc.tile_pool(name="rcp", bufs=3))
        corrpool = cctx.enter_context(tc.tile_pool(name="corr", bufs=1))
        ca = corrpool.tile([117, 34, 66], f32r, name="ca")
        cb = corrpool.tile([52, 34, 66], f32r, name="cb")
        memset_border(ca, 117, 34, 66, 1, 1, 32, 64)
        memset_border(cb, 52, 34, 66, 1, 1, 32, 64)
        Rd = dram.tile([32, 64, ND * 76], f32, name="Rd")
        ident = corrpool.tile([64, 64], f32, name="ident")
        from concourse.masks import make_identity

        make_identity(nc, ident)
        l2flat = low2.rearrange("c a b -> c (a b)")
        for y in range(32):
            pR = psumR.tile([64, ND * 76], mybir.dt.float32, name="pR")
            base = y * 76
            nc.tensor.matmul(
                pR[:, 0:512],
                low1[:, 6 + y, 6:70],
                l2flat[:, base : base + 512],
                start=True,
                stop=True,
            )
            nc.tensor.matmul(
                pR[:, 512 : ND * 76],
                low1[:, 6 + y, 6:70],
                l2flat[:, base + 512 : base + ND * 76],
                start=True,
                stop=True,
            )
            rt = rpool.tile([64, ND * 76], f32, name="rt")
            act(rt, pR, AF.Identity, scale=inv_sqrt_c)
            # R row block to DRAM (contiguous per partition)
            rdst = bass.AP(
                tensor=Rd.tensor,
                offset=Rd.offset + y * 64 * ND * 76,
                ap=[[ND * 76, 64], [1, ND * 76]],
            )
            nc.sync.dma_start(out=rdst, in_=rt)
            # band gather: sh[x1, dy, dx] = R[y, x1, 76*dy + x1 + dx]
            sh = rpool.tile([64, 13, 13], f32, name="sh")
            gsrc = bass.AP(
                tensor=Rd.tensor,
                offset=Rd.offset + y * 64 * ND * 76,
                ap=[[ND * 76 + 1, 64], [76, 13], [1, 13]],
            )
            nc.sync.dma_start(out=sh, in_=gsrc)
            sh = sh.rearrange("p a b -> p (a b)")
            # transpose to channel-major via PE, copy into corr tiles
            pT = psum.tile([128, 2, 256], mybir.dt.float32, name="cp")
            nc.tensor.transpose(pT[:117, 0, 0:64], sh[:, 0:117], ident)
            act(ca[:, 1 + y, 1:65], pT[:117, 0, 0:64], AF.Identity)
            pT2 = psum.tile([128, 2, 256], mybir.dt.float32, name="cp")
            nc.tensor.transpose(pT2[:52, 0, 0:64], sh[:, 117:169], ident)
            act(cb[:, 1 + y, 1:65], pT2[:52, 0, 0:64], AF.Identity)

        # attention: att = sigmoid(W^T corr + b); corr *= att
        wa_a = wpool.tile([117, 169], f32r, name="w")
        nc.sync.dma_start(out=wa_a, in_=P["att_w"][0:117, :].bitcast(f32r))
        wa_b = wpool.tile([64, 169], f32r, name="w")
        nc.sync.dma_start(out=wa_b[:52], in_=P["att_w"][117:169, :].bitcast(f32r))
        ab_a = vecs.tile([117, 1], f32, name="att_ba")
        ab_b = vecs.tile([52, 1], f32, name="att_bb")
        src = P["att_b"][:]
        nc.sync.dma_start(
            out=ab_a,
            in_=bass.AP(tensor=src.tensor, offset=src.offset, ap=[[1, 117], [0, 1]]),
        )
        nc.sync.dma_start(
            out=ab_b,
            in_=bass.AP(tensor=src.tensor, offset=src.offset + 117, ap=[[1, 52], [0, 1]]),
        )
        att_a = corrpool.tile([117, 32, 64], f32, name="att_a")
        att_b = corrpool.tile([52, 32, 64], f32, name="att_b")
        caf = ca.rearrange("c a b -> c (a b)")
        cbf = cb.rearrange("c a b -> c (a b)")
        for mi, (m0, mb, at_, abias) in enumerate(
            [(0, 117, att_a, ab_a), (117, 52, att_b, ab_b)]
        ):
            for y0 in range(0, 32, 8):
                pa = psum.tile([128, 8, 64], mybir.dt.float32, name="cp")
                nc.tensor.matmul(
                    pa[:mb],
                    wa_a[0:117, m0 : m0 + mb],
                    ca[:, 1 + y0 : 9 + y0, 1:65],
                    start=True,
                    stop=False,
                )
                nc.tensor.matmul(
                    pa[:mb],
                    wa_b[:52, m0 : m0 + mb],
                    cb[:, 1 + y0 : 9 + y0, 1:65],
                    start=False,
                    stop=True,
                )
                act(at_[:, y0 : y0 + 8, :], pa[:mb], AF.Sigmoid, bias=abias)
        # gate in place
        nc.vector.tensor_tensor(
            out=ca[:, 1:33, 1:65], in0=ca[:, 1:33, 1:65].bitcast(f32), in1=att_a, op=ALU.mult
        )
        nc.vector.tensor_tensor(
            out=cb[:, 1:33, 1:65], in0=cb[:, 1:33, 1:65].bitcast(f32), in1=att_b, op=ALU.mult
        )

        # cc convs
        ccpool = cctx.enter_context(tc.tile_pool(name="cc", bufs=1))
        cb1p = [None, None]
        key = ("cc1_b", "pair")
        t1 = vecs.tile([128, 1], f32, name="bv_cc1_0")
        t2 = vecs.tile([128, 1], f32, name="bv_cc1_1")
        src = P["cc1_b"][:]
        nc.sync.dma_start(out=t1, in_=bass.AP(tensor=src.tensor, offset=src.offset, ap=[[1, 128], [0, 1]]))
        nc.sync.dma_start(out=t2, in_=bass.AP(tensor=src.tensor, offset=src.offset + 128, ap=[[1, 128], [0, 1]]))
        cc1a = ccpool.tile([128, 34, 66], f32r, name="cc1a")
        cc1b = ccpool.tile([128, 34, 66], f32r, name="cc1b")
        memset_border(cc1a, 128, 34, 66, 1, 1, 32, 64)
        memset_border(cc1b, 128, 34, 66, 1, 1, 32, 64)
        conv3(
            "cc1",
            [(ca, 1, 1), (cb, 1, 1)],
            [117, 52],
            "cc1_w",
            256,
            32,
            64,
            8,
            post_act([(cc1a, 1, 1), (cc1b, 1, 1)], AF.Relu, btile=[t1, t2], W=64),
        )
        ccb2 = bias_of("cc2_b", 128)
        cc2t = ccpool.tile([128, 34, 66], f32r, name="cc2t")
        memset_border(cc2t, 128, 34, 66, 1, 1, 32, 64)
        conv3(
            "cc2",
            [(cc1a, 1, 1), (cc1b, 1, 1)],
            [128, 128],
            "cc2_w",
            128,
            32,
            64,
            8,
            post_act([(cc2t, 1, 1)], AF.Relu, btile=ccb2, W=64),
        )
        ccb3 = bias_of("cc3_b", 64)
        cc3t = ccpool.tile([64, 32, 64], f32, name="cc3t")
        conv3(
            "cc3",
            [(cc2t, 1, 1)],
            [128],
            "cc3_w",
            64,
            32,
            64,
            8,
            post_act([(cc3t, 0, 0)], AF.Relu, btile=ccb3, W=64),
        )
        cfsum = vecs.tile([64, 1], f32, name="cfsum")
        with nc.allow_low_precision(reason="f32r out, fp32 bits"):
            nc.vector.tensor_reduce(cfsum, cc3t, AX.XY, ALU.add)

    # ---- heads ------------------------------------------------------------
    r1, r2 = feats[0]["rsum"], feats[1]["rsum"]
    gc1v = feats[0]["gcv"]

    def matvec(wname, kblocks, M, name):
        """kblocks: list of (vec_tile, kc). Returns psum [M,1] list per m-chunk."""
        wts = []
        k0 = 0
        for j, (v, kc) in enumerate(kblocks):
            t = wpool.tile([128, max(M, 8)], f32, name="w")
            nc.sync.dma_start(out=t[:kc, :M], in_=P[wname][k0 : k0 + kc, :])
            wts.append(t)
            k0 += kc
        outs = []
        for m0 in range(0, M, 128):
            mb = min(128, M - m0)
            pv = psum.tile([128, 2, 256], mybir.dt.float32, name="cp")
            for j, (v, kc) in enumerate(kblocks):
                nc.tensor.matmul(
                    pv[:mb, 0, 0:1],
                    wts[j][:kc, m0 : m0 + mb],
                    v[:kc],
                    start=(j == 0),
                    stop=(j == len(kblocks) - 1),
                )
            outs.append(pv)
        return outs

    # scale branch: sb1(relu) -> sb2(relu) -> sb3 -> softplus
    sb1b = bias_of("sb1_b", 128)
    pv = matvec("sb1_w", [(r1, 128), (r2, 128)], 128, "wsb1")[0]
    s1 = vecs.tile([128, 1], f32, name="s1")
    act(s1, pv[:, 0, 0:1], AF.Relu, bias=sb1b)
    sb2b = bias_of("sb2_b", 64)
    pv = matvec("sb2_w", [(s1, 128)], 64, "wsb2")[0]
    s2 = vecs.tile([64, 1], f32, name="s2")
    act(s2[:64], pv[:64, 0, 0:1], AF.Relu, bias=sb2b)
    sb3b = bias_of("sb3_b", 1)
    pv = matvec("sb3_w", [(s2, 64)], 1, "wsb3")[0]
    sc = vecs.tile([1, 1], f32, name="sc")
    act(sc[:1], pv[:1, 0, 0:1], AF.Exp, bias=sb3b)
    nc.vector.tensor_scalar_add(sc[:1], sc[:1], 1.0)
    act(sc[:1], sc[:1], AF.Ln)
    # broadcast scale to 3 partitions via DRAM bounce
    scd = dram.tile([1], f32, name="scd")
    nc.sync.dma_start(out=scd, in_=sc[0, :])
    sc3 = vecs.tile([3, 1], f32, name="sc3")
    nc.sync.dma_start(
        out=sc3, in_=bass.AP(tensor=scd.tensor, offset=scd.offset, ap=[[0, 3], [1, 1]])
    )

    # trans branch: fc1(relu) -> fc2(relu) -> fc3 -> * scale
    fb1 = [None, None, None, None]
    fc1bs = []
    src = P["fc1_b"][:]
    for j in range(4):
        t = vecs.tile([128, 1], f32, name=f"bv_fc1_{j}")
        nc.sync.dma_start(
            out=t,
            in_=bass.AP(tensor=src.tensor, offset=src.offset + 128 * j, ap=[[1, 128], [0, 1]]),
        )
        fc1bs.append(t)
    pvs = matvec("fc1_w", [(cfsum, 64), (r1, 128), (r2, 128), (gc1v, 128)], 512, "wfc1")
    t1s = []
    for j, pv in enumerate(pvs):
        tt = vecs.tile([128, 1], f32, name=f"t1_{j}")
        act(tt, pv[:, 0, 0:1], AF.Relu, bias=fc1bs[j])
        t1s.append(tt)
    fc2bs = []
    src = P["fc2_b"][:]
    for j in range(2):
        t = vecs.tile([128, 1], f32, name=f"bv_fc2_{j}")
        nc.sync.dma_start(
            out=t,
            in_=bass.AP(tensor=src.tensor, offset=src.offset + 128 * j, ap=[[1, 128], [0, 1]]),
        )
        fc2bs.append(t)
    pvs = matvec("fc2_w", [(t, 128) for t in t1s], 256, "wfc2")
    t2s = []
    for j, pv in enumerate(pvs):
        tt = vecs.tile([128, 1], f32, name=f"t2_{j}")
        act(tt, pv[:, 0, 0:1], AF.Relu, bias=fc2bs[j])
        t2s.append(tt)
    fc3b = bias_of("fc3_b", 3)
    pv = matvec("fc3_w", [(t, 128) for t in t2s], 3, "wfc3")[0]
    tr = vecs.tile([3, 1], f32, name="tr")
    act(tr[:3], pv[:3, 0, 0:1], AF.Identity, bias=fc3b)
    nc.vector.tensor_tensor(out=tr[:3], in0=tr[:3], in1=sc3[:3], op=ALU.mult)
    nc.sync.dma_start(out=out_trans[:], in_=tr[:3, 0])

    ctx.close()
    nc.compile()
    return nc


# ----------------------------------------------------------------------------
# public entry point
# ----------------------------------------------------------------------------
def kernel(img1, img2, params):
    from concourse.bass_utils import run_bass_kernel_spmd

    img1 = _np(img1)
    img2 = _np(img2)
    W = _prep_weights(params)

    if "nc" not in _CACHE:
        _CACHE["nc"] = _build()
    nc = _CACHE["nc"]

    core_ids = list(range(8))
    in_maps = []
    for i in core_ids:
        m = dict(W)
        m["img1"] = np.ascontiguousarray(img1[i])
        m["img2"] = np.ascontiguousarray(img2[i])
        in_maps.append(m)
    r = run_bass_kernel_spmd(nc, in_maps, core_ids)
    trans = np.stack([r.results[i]["trans"] for i in range(8)]).astype(np.float32)
    rot = np.zeros((8, 4), np.float32)
    rot[:, 0] = 1.0
    return rot, trans


# revision 22
# speedup vs baseline: 1.1631x; 1.1631x over previous
"""Trainium2 Bass kernel for EnhancedVisualOdometryModel.

Data-parallel: 8 samples -> 8 NeuronCores, one sample per core.
All conv compute in float32r (TF32-like, 1 cyc/row on PE), fp32 accumulation.
Convs = shifted-view matmuls over padded SBUF feature tiles.
Correlation = per-row GEMM (x1_row^T @ x2_rows band) -> DRAM bounce -> affine
shear-gather DMAs into channel-major cost volume.
"""

import numpy as np

EPS = 1e-5
MD = 6  # max displacement
ND = 2 * MD + 1  # 13
B, H0, W0 = 8, 256, 512

_CACHE = {}


# ----------------------------------------------------------------------------
# host-side weight preparation
# ----------------------------------------------------------------------------
def _np(x):
    return np.asarray(x, dtype=np.float32)


def _bng(g):
    return _np(g) / np.sqrt(np.float32(1.0 + EPS))


def _convT(w, oscale=None):
    """w [O,I,kh,kw] -> lhsT [I, kh*kw, O], optionally scaling output channels."""
    w = _np(w)
    if oscale is not None:
        w = w * oscale[:, None, None, None]
    O, I, kh, kw = w.shape
    return np.ascontiguousarray(w.transpose(1, 2, 3, 0).reshape(I, kh * kw, O))


def _linT(w, oscale=None, iscale=None):
    """w [O,I] -> lhsT [I,O] with optional per-output / per-input scaling."""
    w = _np(w)
    if oscale is not None:
        w = w * oscale[:, None]
    if iscale is not None:
        w = w * iscale[None, :]
    return np.ascontiguousarray(w.T)


# conv1 piece table: (qy, sy, ry0, nry) with dy = 4*sy + ry - 2*qy
# imx partition layout: p = ry*32 + ic*7 + dx (rows 21..31 of each ry block junk)
# piece lhsT rows zero-padded to match; base partition = 32*ry0, K = 32*(nry-1)+21
C1_PIECES = [
    (0, 0, 0, 4),  # base 0,  K 117
    (0, 1, 0, 3),  # base 0,  K 85
    (1, 0, 2, 2),  # base 64, K 53
    (1, 1, 0, 4),  # base 0,  K 117
    (1, 2, 0, 1),  # base 0,  K 21
]
C1_KP = [32 * (n - 1) + 21 for (_, _, _, n) in C1_PIECES]
C1_BASE = [32 * r0 for (_, _, r0, _) in C1_PIECES]
C1_ROWS = sum(C1_KP)  # 393


def _prep_conv1(w):  # w [64,3,7,7] -> [C1_ROWS, 64]
    w = _np(w)
    rows = []
    for pi, (qy, sy, ry0, nry) in enumerate(C1_PIECES):
        blk = np.zeros((C1_KP[pi], 64), np.float32)
        for j in range(nry):
            ry = ry0 + j
            dy = 4 * sy + ry - 2 * qy
            for ic in range(3):
                for dx in range(7):
                    blk[j * 32 + ic * 7 + dx] = w[:, ic, dy, dx]
        rows.append(blk)
    return np.concatenate(rows, axis=0)


def _emb169(idx_o, idx_i, w):
    """embed [168-ch] conv weights into 169-ch (dy*13+dx) layout with center hole."""
    # orig channel order: (i,j) for i in -6..6, j in -6..6, skipping (0,0)
    # -> orig index o: flat = (i+6)*13 + (j+6); o = flat - (flat > 84)
    return None


def _o169_map():
    m = np.zeros(169, np.int64)
    valid = np.ones(169, bool)
    k = 0
    for f in range(169):
        if f == 84:
            valid[f] = False
            m[f] = 0
        else:
            m[f] = k
            k += 1
    return m, valid


def _prep_weights(p):
    m169, v169 = _o169_map()
    W = {}

    W["w_c1"] = _prep_conv1(p["conv1_w"])
    W["c1_g"] = _bng(p["bn1_g"])
    W["c1_b"] = _np(p["bn1_b"])

    for li, layer in enumerate(["layer1", "layer2", "layer3"]):
        for bi, blk in enumerate(p[layer]):
            pre = f"l{li + 1}{bi}"
            g1, b1 = _bng(blk["g1"]), _np(blk["b1"])
            g2, b2 = _bng(blk["g2"]), _np(blk["b2"])
            W[f"{pre}_w1"] = _convT(blk["c1"])
            W[f"{pre}_g1"] = g1
            W[f"{pre}_b1"] = b1
            W[f"{pre}_w2"] = _convT(blk["c2"], oscale=g2)  # g2 folded
            W[f"{pre}_b2"] = b2
            if "cd" in blk:
                gd, bd = _bng(blk["gd"]), _np(blk["bd"])
                W[f"{pre}_wd"] = _linT(_np(blk["cd"])[:, :, 0, 0], oscale=gd)
                W[f"{pre}_bd"] = bd

    g = _bng(p["ref1_g"])
    W["ref1_w"] = _convT(p["ref1_w"], oscale=g)
    W["ref1_b"] = _np(p["ref1_b"]) * g + _np(p["ref1_bb"])
    g = _bng(p["ref2_g"])
    W["ref2_w"] = _convT(p["ref2_w"], oscale=g)
    W["ref2_b"] = _np(p["ref2_b"]) * g + _np(p["ref2_bb"])

    W["gc1_w"] = _linT(_np(p["gc1_w"])[:, :, 0, 0], iscale=np.full(256, 1.0 / 512, np.float32))
    W["gc1_b"] = _np(p["gc1_b"])
    W["gc2_w"] = _linT(_np(p["gc2_w"])[:, :, 0, 0])
    W["gc2_b"] = _np(p["gc2_b"])

    # attention 1x1: embed 168 -> 169
    aw = _np(p["att_w"])[:, :, 0, 0]  # [168, 168]
    a169 = np.zeros((169, 169), np.float32)
    io = np.where(v169)[0]
    a169[np.ix_(io, io)] = aw[np.ix_(m169[io], m169[io])].T  # lhsT [in169, out169]
    W["att_w"] = a169
    ab = np.zeros(169, np.float32)
    ab[io] = _np(p["att_b"])[m169[io]]
    W["att_b"] = ab

    # cc1: [256, 168, 3, 3] -> lhsT [169, 9, 256]
    g = _bng(p["cc1_g"])
    c1t = _convT(p["cc1_w"], oscale=g)  # [168, 9, 256]
    cc1 = np.zeros((169, 9, 256), np.float32)
    cc1[io] = c1t[m169[io]]
    W["cc1_w"] = cc1
    W["cc1_b"] = _np(p["cc1_b"]) * g + _np(p["cc1_bb"])
    g = _bng(p["cc2_g"])
    W["cc2_w"] = _convT(p["cc2_w"], oscale=g)
    W["cc2_b"] = _np(p["cc2_b"]) * g + _np(p["cc2_bb"])
    g = _bng(p["cc3_g"])
    W["cc3_w"] = _convT(p["cc3_w"], oscale=g)
    W["cc3_b"] = _np(p["cc3_b"]) * g + _np(p["cc3_bb"])

    g = _bng(p["sb1_g"])
    W["sb1_w"] = _linT(p["sb1_w"], oscale=g, iscale=np.full(256, 1.0 / 512, np.float32))
    W["sb1_b"] = _np(p["sb1_b"]) * g + _np(p["sb1_bb"])
    g = _bng(p["sb2_g"])
    W["sb2_w"] = _linT(p["sb2_w"], oscale=g)
    W["sb2_b"] = _np(p["sb2_b"]) * g + _np(p["sb2_bb"])
    W["sb3_w"] = _linT(p["sb3_w"])
    W["sb3_b"] = _np(p["sb3_b"])

    isc = np.concatenate(
        [
            np.full(64, 1.0 / 2048, np.float32),
            np.full(128, 1.0 / 512, np.float32),
            np.full(128, 1.0 / 512, np.float32),
            np.ones(128, np.float32),
        ]
    )
    g = _bng(p["fc1_g"])
    W["fc1_w"] = _linT(p["fc1_w"], oscale=g, iscale=isc)
    W["fc1_b"] = _np(p["fc1_b"]) * g + _np(p["fc1_bb"])
    g = _bng(p["fc2_g"])
    W["fc2_w"] = _linT(p["fc2_w"], oscale=g)
    W["fc2_b"] = _np(p["fc2_b"]) * g + _np(p["fc2_bb"])
    W["fc3_w"] = _linT(p["fc3_w"])
    W["fc3_b"] = _np(p["fc3_b"])
    return W


# ----------------------------------------------------------------------------
# device program
# ----------------------------------------------------------------------------
def _build():
    import concourse.bass as bass
    import concourse.mybir as mybir
    import concourse.tile as tile
    from concourse import bacc
    from contextlib import ExitStack

    f32 = mybir.dt.float32
    f32r = mybir.dt.float32r
    AF = mybir.ActivationFunctionType
    ALU = mybir.AluOpType
    AX = mybir.AxisListType

    nc = bacc.Bacc(None)

    # ---- parameters -------------------------------------------------------
    def par(name, shape):
        return nc.declare_dram_parameter(name, list(shape), f32, isOutput=False)

    P = {}
    P["img1"] = par("img1", (3, 256, 512))
    P["img2"] = par("img2", (3, 256, 512))
    P["w_c1"] = par("w_c1", (C1_ROWS, 64))
    P["c1_g"] = par("c1_g", (64,))
    P["c1_b"] = par("c1_b", (64,))
    for pre, ic, oc, down in [
        ("l10", 64, 64, False),
        ("l11", 64, 64, False),
        ("l20", 64, 128, True),
        ("l21", 128, 128, False),
        ("l30", 128, 256, True),
        ("l31", 256, 256, False),
    ]:
        P[f"{pre}_w1"] = par(f"{pre}_w1", (ic, 9, oc))
        P[f"{pre}_g1"] = par(f"{pre}_g1", (oc,))
        P[f"{pre}_b1"] = par(f"{pre}_b1", (oc,))
        P[f"{pre}_w2"] = par(f"{pre}_w2", (oc, 9, oc))
        P[f"{pre}_b2"] = par(f"{pre}_b2", (oc,))
        if down:
            P[f"{pre}_wd"] = par(f"{pre}_wd", (ic, oc))
            P[f"{pre}_bd"] = par(f"{pre}_bd", (oc,))
    P["ref1_w"] = par("ref1_w", (256, 9, 256))
    P["ref1_b"] = par("ref1_b", (256,))
    P["ref2_w"] = par("ref2_w", (256, 9, 128))
    P["ref2_b"] = par("ref2_b", (128,))
    P["gc1_w"] = par("gc1_w", (256, 128))
    P["gc1_b"] = par("gc1_b", (128,))
    P["gc2_w"] = par("gc2_w", (128, 128))
    P["gc2_b"] = par("gc2_b", (128,))
    P["att_w"] = par("att_w", (169, 169))
    P["att_b"] = par("att_b", (169,))
    P["cc1_w"] = par("cc1_w", (169, 9, 256))
    P["cc1_b"] = par("cc1_b", (256,))
    P["cc2_w"] = par("cc2_w", (256, 9, 128))
    P["cc2_b"] = par("cc2_b", (128,))
    P["cc3_w"] = par("cc3_w", (128, 9, 64))
    P["cc3_b"] = par("cc3_b", (64,))
    P["sb1_w"] = par("sb1_w", (256, 128))
    P["sb1_b"] = par("sb1_b", (128,))
    P["sb2_w"] = par("sb2_w", (128, 64))
    P["sb2_b"] = par("sb2_b", (64,))
    P["sb3_w"] = par("sb3_w", (64, 1))
    P["sb3_b"] = par("sb3_b", (1,))
    P["fc1_w"] = par("fc1_w", (448, 512))
    P["fc1_b"] = par("fc1_b", (512,))
    P["fc2_w"] = par("fc2_w", (512, 256))
    P["fc2_b"] = par("fc2_b", (256,))
    P["fc3_w"] = par("fc3_w", (256, 3))
    P["fc3_b"] = par("fc3_b", (3,))

    out_trans = nc.declare_dram_parameter("trans", [3], f32, isOutput=True)

    ctx = ExitStack()
    tc = ctx.enter_context(tile.TileContext(nc))

    vecs = ctx.enter_context(tc.tile_pool(name="vecs", bufs=1))
    wpool = ctx.enter_context(tc.tile_pool(name="wpool", bufs=3))
    psum = ctx.enter_context(tc.tile_pool(name="psum", bufs=4, space="PSUM"))
    psumR = ctx.enter_context(tc.tile_pool(name="psumR", bufs=2, space="PSUM"))
    dram = ctx.enter_context(tc.tile_pool(name="dram", bufs=1, space="DRAM"))

    # ---- small helpers ----------------------------------------------------
    def vec_tile(name, param, n, dt=f32):
        t = vecs.tile([n, 1], dt, name=name)
        src = param[:]
        ap = bass.AP(tensor=src.tensor, offset=src.offset, ap=[[1, n], [0, 1]])
        nc.sync.dma_start(out=t, in_=ap.bitcast(dt))
        return t

    BIAS = {}

    def bias_of(pname, n, dt=f32):
        key = (pname, n, dt)
        if key not in BIAS:
            BIAS[key] = vec_tile(f"bv_{pname}_{n}", P[pname], n, dt)
        return BIAS[key]

    def load_w(pname, shape, name=None, pool=None):
        """DMA a weight param (or slice) into an f32r SBUF tile."""
        pl = pool or wpool
        t = pl.tile(list(shape), f32r, name=name or "w")
        ap = P[pname][:]
        nc.sync.dma_start(out=t, in_=ap.bitcast(f32r))
        return t

    def act(out_ap, in_ap, func, bias=0.0, scale=1.0):
        nc.scalar.activation(out_ap, in_ap, func, bias=bias, scale=scale)

    # generic 3x3 conv over padded tiles --------------------------------
    # src_blocks: list of (tile, oy, ox) each holding <=128 channels
    # kcs: channel count per src block; w param name with shape [ICtot, 9, OCtot]
    def conv3(
        name,
        src_blocks,
        kcs,
        wname,
        octot,
        H,
        W,
        CR,
        post,
        stride=1,
    ):
        ictot = sum(kcs)
        wts = []
        k0 = 0
        for i, kc in enumerate(kcs):
            t = wpool.tile([128, 9, octot], f32r, name="w")
            ap = P[wname][k0 : k0 + kc, :, :]
            nc.sync.dma_start(out=t[:kc], in_=ap.bitcast(f32r))
            wts.append(t)
            k0 += kc
        mlist = [(m0, min(128, octot - m0)) for m0 in range(0, octot, 128)]
        for mi, (m0, mb) in enumerate(mlist):
            for y0 in range(0, H, CR):
                rows = min(CR, H - y0)
                p = psum.tile([128, CR, W], mybir.dt.float32, name="cp")
                first = True
                nmm = len(kcs) * 9
                k = 0
                for ki, kc in enumerate(kcs):
                    st, soy, sox = src_blocks[ki]
                    for t in range(9):
                        ty, tx = divmod(t, 3)
                        r0 = soy + stride * y0 + ty - 1
                        c0 = sox + tx - 1
                        rhs = st[
                            :kc,
                            r0 : r0 + stride * rows : stride,
                            c0 : c0 + stride * W : stride,
                        ]
                        k += 1
                        nc.tensor.matmul(
                            p[:mb, :rows, :],
                            wts[ki][:kc, t, m0 : m0 + mb],
                            rhs,
                            start=first,
                            stop=(k == nmm),
                        )
                        first = False
                post(p, mi, m0, mb, y0, rows)

    def post_act(dsts, func, gtile=None, btile=None, W=None):
        """dsts: list of (tile, oy, ox) per M-chunk."""

        def post(p, mi, m0, mb, y0, rows):
            dt_, oy, ox = dsts[mi]
            sc = gtile[mi] if isinstance(gtile, list) else (gtile if gtile is not None else 1.0)
            bi = btile[mi] if isinstance(btile, list) else (btile if btile is not None else 0.0)
            act(
                dt_[:mb, oy + y0 : oy + y0 + rows, ox : ox + W],
                p[:mb, :rows, :],
                func,
                bias=bi,
                scale=sc,
            )

        return post

    def post_res(dsts, idns, btile, W):
        """residual: dst = relu(psum + idn + b). idns: (tile, oy, ox) per chunk."""

        def post(p, mi, m0, mb, y0, rows):
            dt_, oy, ox = dsts[mi]
            it_, ioy, iox = idns[mi]
            dv = dt_[:mb, oy + y0 : oy + y0 + rows, ox : ox + W]
            nc.vector.tensor_tensor(
                out=dv,
                in0=p[:mb, :rows, :],
                in1=it_[:mb, ioy + y0 : ioy + y0 + rows, iox : iox + W],
                op=ALU.add,
            )
            b = btile[mi] if isinstance(btile, list) else btile
            act(dv, dv.bitcast(f32), AF.Relu, bias=b)

        return post

    def memset_border(t, C, Hp, Wp, oy, ox, H, W, val=0.0):
        for ap in [
            t[:C, 0:oy, :] if oy > 0 else None,
            t[:C, oy + H : Hp, :] if oy + H < Hp else None,
            t[:C, :, 0:ox] if ox > 0 else None,
            t[:C, :, ox + W : Wp] if ox + W < Wp else None,
        ]:
            if ap is not None:
                nc.vector.memset(ap.bitcast(f32), val)

    def conv1x1_s2(src, soy, sox, kc, wt, octot, H, W, CR, post):
        mlist = [(m0, min(128, octot - m0)) for m0 in range(0, octot, 128)]
        for mi, (m0, mb) in enumerate(mlist):
            for y0 in range(0, H, CR):
                rows = min(CR, H - y0)
                p = psum.tile([128, CR, W], mybir.dt.float32, name="cp")
                rhs = src[
                    :kc,
                    soy + 2 * y0 : soy + 2 * y0 + 2 * rows : 2,
                    sox : sox + 2 * W : 2,
                ]
                nc.tensor.matmul(
                    p[:mb, :rows, :], wt[:kc, m0 : m0 + mb], rhs, start=True, stop=True
                )
                post(p, mi, m0, mb, y0, rows)

    # residual basic block (stride 1), in-place: io tile holds in & out
    def basic_block(pre, io, C, H, W, mpool, mtag):
        g1 = bias_of(f"{pre}_g1", C)
        b1 = bias_of(f"{pre}_b1", C)
        b2 = bias_of(f"{pre}_b2", C)
        m = mpool.tile([C, H + 2, W + 2], f32r, name=mtag)
        memset_border(m, C, H + 2, W + 2, 1, 1, H, W)
        CR = max(1, 512 // W)
        conv3(
            f"{pre}c1",
            [(io, 1, 1)],
            [C],
            f"{pre}_w1",
            C,
            H,
            W,
            CR,
            post_act([(m, 1, 1)], AF.Relu, gtile=g1, btile=b1, W=W),
        )
        conv3(
            f"{pre}c2",
            [(m, 1, 1)],
            [C],
            f"{pre}_w2",
            C,
            H,
            W,
            CR,
            post_res([(io, 1, 1)], [(io, 1, 1)], b2, W),
        )

    def zero_dram(zt, dt_, nelem):
        # dt_ flat [nelem], nelem divisible by 2145; zeros in [128,2145] chunks
        step = 128 * 2145
        off = 0
        while off < nelem:
            n = min(step, nelem - off)
            rows = n // 2145
            dst = bass.AP(
                tensor=dt_.tensor,
                offset=dt_.offset + off,
                ap=[[2145, rows], [1, 2145]],
            )
            nc.sync.dma_start(out=dst, in_=zt[:rows, :])
            off += rows * 2145

    # ---- per-image feature extraction -------------------------------------
    pool1pool = ctx.enter_context(tc.tile_pool(name="pool1pool", bufs=1))
    lowpads = ctx.enter_context(tc.tile_pool(name="lowpads", bufs=2))
    feats = {}

    EV_H, EV_W = 264, 260

    for ii, iname in enumerate(["img1", "img2"]):
        img = P[iname]
        with ExitStack() as imctx:
            # --- phase A: conv1 + maxpool ---
            actx = imctx.enter_context(ExitStack())
            prectx = ExitStack()
            mpoolz = prectx.enter_context(tc.tile_pool(name=f"zt_{ii}", bufs=1))
            zt = mpoolz.tile([128, 2145], f32, name="zt")
            nc.vector.memset(zt, 0.0)
            evb = dram.tile([3 * EV_H * EV_W], f32, name=f"evb{ii}")
            odb = dram.tile([3 * EV_H * EV_W], f32, name=f"odb{ii}")
            zero_dram(zt, evb, 3 * EV_H * EV_W)
            zero_dram(zt, odb, 3 * EV_H * EV_W)
            evs = [
                bass.AP(tensor=evb.tensor, offset=evb.offset + ic * EV_H * EV_W, ap=[[1, 1]])
                for ic in range(3)
            ]
            ods = [
                bass.AP(tensor=odb.tensor, offset=odb.offset + ic * EV_H * EV_W, ap=[[1, 1]])
                for ic in range(3)
            ]

            # load image rows onto partitions: I[p, n, w] = img[row p*8+n]
            it = mpoolz.tile([128, 8, 512], f32, name="imgrows")
            iap = img[:, :, :]
            srcv = bass.AP(
                tensor=iap.tensor,
                offset=iap.offset,
                ap=[[8 * 512, 96], [512, 8], [1, 512]],
            )
            nc.sync.dma_start(out=it[:96], in_=srcv)
            # split phases on DVE
            et = mpoolz.tile([128, 8, 256], f32, name="evsplit")  # img even cols
            ot = mpoolz.tile([128, 8, 256], f32, name="odsplit")  # img odd cols
            nc.vector.tensor_copy(et[:96], it[:96, :, 0:512:2])
            nc.vector.tensor_copy(ot[:96], it[:96, :, 1:512:2])
            # even_img[r, u] = img[r, 2u-3] (odd cols, u in [2,257])
            # odd_img[r, u]  = img[r, 2u-2] (even cols, u in [1,256])
            for ic in range(3):
                dst = bass.AP(
                    tensor=evb.tensor,
                    offset=evb.offset + ic * EV_H * EV_W + 3 * EV_W + 2,
                    ap=[[8 * EV_W, 32], [EV_W, 8], [1, 256]],
                )
                nc.sync.dma_start(out=dst, in_=ot[32 * ic : 32 * ic + 32])
                dst = bass.AP(
                    tensor=odb.tensor,
                    offset=odb.offset + ic * EV_H * EV_W + 3 * EV_W + 1,
                    ap=[[8 * EV_W, 32], [EV_W, 8], [1, 256]],
                )
                nc.sync.dma_start(out=dst, in_=et[32 * ic : 32 * ic + 32])

            prectx.close()
            pool1 = pool1pool.tile([64, 66, 130], f32r, name="pool1")
            memset_border(pool1, 64, 66, 130, 1, 1, 64, 128)

            c1g = bias_of("c1_g", 64)
            c1b = bias_of("c1_b", 64)
            wc1pool = actx.enter_context(tc.tile_pool(name=f"wc1_{ii}", bufs=5))
            wc1 = []
            r0 = 0
            for pi in range(5):
                K = C1_KP[pi]
                bse = C1_BASE[pi]
                t = wc1pool.tile([117, 64], f32r, name="wc1p")
                nc.sync.dma_start(
                    out=t[bse : bse + K], in_=P["w_c1"][r0 : r0 + K, :].bitcast(f32r)
                )
                wc1.append((t, K))
                r0 += K

            impool = actx.enter_context(tc.tile_pool(name=f"imx_{ii}", bufs=2))
            slabpool = actx.enter_context(tc.tile_pool(name=f"c1s_{ii}", bufs=2))
            tvpool = actx.enter_context(tc.tile_pool(name=f"tv_{ii}", bufs=1))

            for s in range(4):
                # imx slab rows Y'' in [Ib, Ib+nrowsY)
                Ib = max(0, 16 * s - 1)
                Ie = min(66, 16 * s + 19)
                nY = Ie - Ib
                imx = impool.tile([128, 20, 256], f32r, name="imx")
                for ry in range(4):
                    for ic in range(3):
                        pbase = ry * 32 + ic * 7
                        # even dx (0,2,4,6): u = x + dx/2 from even plane
                        src = bass.AP(
                            tensor=evs[ic].tensor,
                            offset=evs[ic].offset + (4 * Ib + ry) * EV_W + 0,
                            ap=[[1, 4], [4 * EV_W, nY], [1, 256]],
                        )
                        nc.sync.dma_start(
                            out=imx[pbase : pbase + 7 : 2, :nY, :],
                            in_=src.bitcast(f32r),
                        )
                        # odd dx (1,3,5): u = x + (dx-1)/2 from odd plane
                        src = bass.AP(
                            tensor=ods[ic].tensor,
                            offset=ods[ic].offset + (4 * Ib + ry) * EV_W + 0,
                            ap=[[1, 3], [4 * EV_W, nY], [1, 256]],
                        )
                        nc.sync.dma_start(
                            out=imx[pbase + 1 : pbase + 6 : 2, :nY, :],
                            in_=src.bitcast(f32r),
                        )

                # c1 slab: c1pad rows [32s, 32s+34) ; c1pad row r = c1 row r-1
                slab = slabpool.tile([64, 34, 258], mybir.dt.bfloat16, name="c1s")
                nc.vector.memset(slab[:, :, 0:1], -1e30)
                nc.vector.memset(slab[:, :, 257:258], -1e30)
                if s == 0:
                    nc.vector.memset(slab[:, 0:1, :], -1e30)
                if s == 3:
                    nc.vector.memset(slab[:, 33:34, :], -1e30)
                rlo = max(0, 32 * s - 1)
                rhi = min(128, 32 * s + 33)
                for qy in (0, 1):
                    Ys = [Y for Y in range(64) if rlo <= 2 * Y + qy < rhi]
                    pieces = [pc for pc in range(5) if C1_PIECES[pc][0] == qy]
                    i = 0
                    while i < len(Ys):
                        # chunk of up to 2 consecutive Y
                        cn = 1
                        if i + 1 < len(Ys) and Ys[i + 1] == Ys[i] + 1:
                            cn = 2
                        Y = Ys[i]
                        p = psum.tile([64, 2, 256], mybir.dt.float32, name="cp")
                        for j, pc in enumerate(pieces):
                            _, sy, ry0, nry = C1_PIECES[pc]
                            K = C1_KP[pc]
                            bse = C1_BASE[pc]
                            wt, _ = wc1[pc]
                            rhs = imx[
                                bse : bse + K, Y + sy - Ib : Y + sy - Ib + cn, :
                            ]
                            nc.tensor.matmul(
                                p[:, :cn, :],
                                wt[C1_BASE[pc] : C1_BASE[pc] + K],
                                rhs,
                                start=(j == 0),
                                stop=(j == len(pieces) - 1),
                            )
                        r = 2 * Y + qy  # first c1 row of chunk; rows r, r+2
                        si = r + 1 - 32 * s  # slab row index
                        act(
                            slab[:, si : si + 2 * cn - 1 : 2, 1:257],
                            p[:, :cn, :],
                            AF.Relu,
                            bias=c1b,
                            scale=c1g,
                        )
                        i += cn

                # maxpool slab -> pool1 rows [1+16s, 1+16s+16)
                tv = tvpool.tile([64, 16, 258], mybir.dt.bfloat16, name="tv")
                nc.vector.tensor_tensor(
                    out=tv, in0=slab[:, 0:32:2, :], in1=slab[:, 1:33:2, :], op=ALU.max
                )
                nc.vector.tensor_tensor(
                    out=tv, in0=tv, in1=slab[:, 2:34:2, :], op=ALU.max
                )
                q0 = 1 + 16 * s
                dstv = pool1[:, q0 : q0 + 16, 1:129]
                nc.vector.tensor_tensor(
                    out=dstv, in0=tv[:, :, 0:256:2], in1=tv[:, :, 1:257:2], op=ALU.max
                )
                nc.vector.tensor_tensor(
                    out=dstv, in0=dstv.bitcast(f32), in1=tv[:, :, 2:258:2], op=ALU.max
                )

            actx.close()

            # --- phase B: layer1 (in-place on pool1) ---
            with ExitStack() as bctx:
                mpool = bctx.enter_context(tc.tile_pool(name=f"l1m_{ii}", bufs=1))
                basic_block("l10", pool1, 64, 64, 128, mpool, "l1m")
                basic_block("l11", pool1, 64, 64, 128, mpool, "l1m")

                # --- phase C: layer2 ---
                l2pool = bctx.enter_context(tc.tile_pool(name=f"l2_{ii}", bufs=1))
                m2pool = bctx.enter_context(tc.tile_pool(name=f"l2m_{ii}", bufs=2))
                # l20: conv1 s2 64->128
                g1 = bias_of("l20_g1", 128)
                b1 = bias_of("l20_b1", 128)
                m2 = m2pool.tile([128, 34, 66], f32r, name="l2m")
                memset_border(m2, 128, 34, 66, 1, 1, 32, 64)
                conv3(
                    "l20c1",
                    [(pool1, 1, 1)],
                    [64],
                    "l20_w1",
                    128,
                    32,
                    64,
                    8,
                    post_act([(m2, 1, 1)], AF.Relu, gtile=g1, btile=b1, W=64),
                    stride=2,
                )
                # downsample idn
                wd = load_w("l20_wd", (64, 128))
                bd = bias_of("l20_bd", 128)
                idn2 = l2pool.tile([128, 32, 64], f32, name="l2idn")
                conv1x1_s2(
                    pool1,
                    1,
                    1,
                    64,
                    wd,
                    128,
                    32,
                    64,
                    8,
                    post_act([(idn2, 0, 0)], AF.Identity, btile=bd, W=64),
                )
                b2 = bias_of("l20_b2", 128)
                io2 = l2pool.tile([128, 34, 66], f32r, name="l2io")
                memset_border(io2, 128, 34, 66, 1, 1, 32, 64)
                conv3(
                    "l20c2",
                    [(m2, 1, 1)],
                    [128],
                    "l20_w2",
                    128,
                    32,
                    64,
                    8,
                    post_res([(io2, 1, 1)], [(idn2, 0, 0)], b2, 64),
                )
                # l21 in-place on io2, but output goes to lowpad
                g1 = bias_of("l21_g1", 128)
                b1 = bias_of("l21_b1", 128)
                b2 = bias_of("l21_b2", 128)
                m2b = m2pool.tile([128, 34, 66], f32r, name="l2m")
                memset_border(m2b, 128, 34, 66, 1, 1, 32, 64)
                conv3(
                    "l21c1",
                    [(io2, 1, 1)],
                    [128],
                    "l21_w1",
                    128,
                    32,
                    64,
                    8,
                    post_act([(m2b, 1, 1)], AF.Relu, gtile=g1, btile=b1, W=64),
                )
                low = lowpads.tile([128, 44, 76], f32r, name="low")
                memset_border(low, 128, 44, 76, 6, 6, 32, 64)
                conv3(
                    "l21c2",
                    [(m2b, 1, 1)],
                    [128],
                    "l21_w2",
                    128,
                    32,
                    64,
                    8,
                    post_res([(low, 6, 6)], [(io2, 1, 1)], b2, 64),
                )

                # --- phase D: layer3 ---
                l3pool = bctx.enter_context(tc.tile_pool(name=f"l3_{ii}", bufs=1))
                # l30 conv1 s2 128->256
                def bias_pair(pname, n):
                    key = (pname, "pair")
                    if key not in BIAS:
                        t1 = vecs.tile([128, 1], f32, name=f"bv_{pname}_0")
                        t2 = vecs.tile([128, 1], f32, name=f"bv_{pname}_1")
                        src = P[pname][:]
                        ap1 = bass.AP(tensor=src.tensor, offset=src.offset, ap=[[1, 128], [0, 1]])
                        ap2 = bass.AP(tensor=src.tensor, offset=src.offset + 128, ap=[[1, 128], [0, 1]])
                        nc.sync.dma_start(out=t1, in_=ap1)
                        nc.sync.dma_start(out=t2, in_=ap2)
                        BIAS[key] = [t1, t2]
                    return BIAS[key]

                g1p = bias_pair("l30_g1", 256)
                b1p = bias_pair("l30_b1", 256)
                m3a = l3pool.tile([128, 18, 34], f32r, name="l3ma")
                m3b = l3pool.tile([128, 18, 34], f32r, name="l3mb")
                for t in (m3a, m3b):
                    memset_border(t, 128, 18, 34, 1, 1, 16, 32)
                conv3(
                    "l30c1",
                    [(low, 6, 6)],
                    [128],
                    "l30_w1",
                    256,
                    16,
                    32,
                    16,
                    post_act([(m3a, 1, 1), (m3b, 1, 1)], AF.Relu, gtile=g1p, btile=b1p, W=32),
                    stride=2,
                )
                wd = load_w("l30_wd", (128, 256))
                bdp = bias_pair("l30_bd", 256)
                idn3a = l3pool.tile([128, 16, 32], f32, name="l3ia")
                idn3b = l3pool.tile([128, 16, 32], f32, name="l3ib")
                conv1x1_s2(
                    low,
                    6,
                    6,
                    128,
                    wd,
                    256,
                    16,
                    32,
                    16,
                    post_act([(idn3a, 0, 0), (idn3b, 0, 0)], AF.Identity, btile=bdp, W=32),
                )
                b2p = bias_pair("l30_b2", 256)
                h_a = l3pool.tile([128, 18, 34], f32r, name="h_a")
                h_b = l3pool.tile([128, 18, 34], f32r, name="h_b")
                for t in (h_a, h_b):
                    memset_border(t, 128, 18, 34, 1, 1, 16, 32)
                conv3(
                    "l30c2",
                    [(m3a, 1, 1), (m3b, 1, 1)],
                    [128, 128],
                    "l30_w2",
                    256,
                    16,
                    32,
                    16,
                    post_res(
                        [(h_a, 1, 1), (h_b, 1, 1)],
                        [(idn3a, 0, 0), (idn3b, 0, 0)],
                        b2p,
                        32,
                    ),
                )
                # l31 (256ch, in-place on h_a/h_b)
                g1p = bias_pair("l31_g1", 256)
                b1p = bias_pair("l31_b1", 256)
                b2p = bias_pair("l31_b2", 256)
                m3c = l3pool.tile([128, 18, 34], f32r, name="l3mc")
                m3d = l3pool.tile([128, 18, 34], f32r, name="l3md")
                for t in (m3c, m3d):
                    memset_border(t, 128, 18, 34, 1, 1, 16, 32)
                conv3(
                    "l31c1",
                    [(h_a, 1, 1), (h_b, 1, 1)],
                    [128, 128],
                    "l31_w1",
                    256,
                    16,
                    32,
                    16,
                    post_act([(m3c, 1, 1), (m3d, 1, 1)], AF.Relu, gtile=g1p, btile=b1p, W=32),
                )
                conv3(
                    "l31c2",
                    [(m3c, 1, 1), (m3d, 1, 1)],
                    [128, 128],
                    "l31_w2",
                    256,
                    16,
                    32,
                    16,
                    post_res(
                        [(h_a, 1, 1), (h_b, 1, 1)],
                        [(h_a, 1, 1), (h_b, 1, 1)],
                        b2p,
                        32,
                    ),
                )

                # gc pooling (sum over spatial of high)
                gsa = vecs.tile([128, 1], f32, name=f"gsa{ii}")
                gsb = vecs.tile([128, 1], f32, name=f"gsb{ii}")
                with nc.allow_low_precision(reason="f32r out, fp32 bits"):
                    nc.vector.tensor_reduce(gsa, h_a[:, 1:17, 1:33], AX.XY, ALU.add)
                    nc.vector.tensor_reduce(gsb, h_b[:, 1:17, 1:33], AX.XY, ALU.add)

                # --- phase E: refinement convs ---
                r1p = bias_pair("ref1_b", 256)
                ra = l3pool.tile([128, 18, 34], f32r, name="ra")
                rb = l3pool.tile([128, 18, 34], f32r, name="rb")
                for t in (ra, rb):
                    memset_border(t, 128, 18, 34, 1, 1, 16, 32)
                conv3(
                    "ref1",
                    [(h_a, 1, 1), (h_b, 1, 1)],
                    [128, 128],
                    "ref1_w",
                    256,
                    16,
                    32,
                    16,
                    post_act([(ra, 1, 1), (rb, 1, 1)], AF.Relu, btile=r1p, W=32),
                )
                r2b = bias_of("ref2_b", 128)
                rr = l3pool.tile([128, 16, 32], f32, name="rr")
                conv3(
                    "ref2",
                    [(ra, 1, 1), (rb, 1, 1)],
                    [128, 128],
                    "ref2_w",
                    128,
                    16,
                    32,
                    16,
                    post_act([(rr, 0, 0)], AF.Relu, btile=r2b, W=32),
                )
                rsum = vecs.tile([128, 1], f32, name=f"rsum{ii}")
                with nc.allow_low_precision(reason="f32r out, fp32 bits"):
                    nc.vector.tensor_reduce(rsum, rr, AX.XY, ALU.add)

                # gc head: gc1 (relu) -> gc2 (sigmoid)
                wg1a = wpool.tile([128, 128], f32, name="w")
                nc.sync.dma_start(out=wg1a, in_=P["gc1_w"][0:128, :])
                wg1b = wpool.tile([128, 128], f32, name="w")
                nc.sync.dma_start(out=wg1b, in_=P["gc1_w"][128:256, :])
                gb1 = bias_of("gc1_b", 128)
                pg = psum.tile([128, 2, 256], mybir.dt.float32, name="cp")
                nc.tensor.matmul(pg[:, 0, 0:1], wg1a, gsa, start=True, stop=False)
                nc.tensor.matmul(pg[:, 0, 0:1], wg1b, gsb, start=False, stop=True)
                gv = vecs.tile([128, 1], f32, name=f"gv{ii}")
                act(gv, pg[:, 0, 0:1], AF.Relu, bias=gb1)
                wg2 = wpool.tile([128, 128], f32, name="w")
                nc.sync.dma_start(out=wg2, in_=P["gc2_w"][:, :])
                gb2 = bias_of("gc2_b", 128)
                pg2 = psum.tile([128, 2, 256], mybir.dt.float32, name="cp")
                nc.tensor.matmul(pg2[:, 0, 0:1], wg2, gv, start=True, stop=True)
                gcv = vecs.tile([128, 1], f32, name=f"gcv{ii}")
                act(gcv, pg2[:, 0, 0:1], AF.Sigmoid, bias=gb2)

            feats[ii] = dict(low=low, rsum=rsum, gcv=gcv)

    # ---- correlation ------------------------------------------------------
    low1 = feats[0]["low"]
    low2 = feats[1]["low"]
    inv_sqrt_c = float(1.0 / np.sqrt(128.0))

    with ExitStack() as cctx:
        rpool = cctx.enter_context(tc.tile_pool(name="rcp", bufs=3))
        corrpool = cctx.enter_context(tc.tile_pool(name="corr", bufs=1))
        ca = corrpool.tile([117, 34, 66], f32r, name="ca")
        cb = corrpool.tile([52, 34, 66], f32r, name="cb")
        memset_border(ca, 117, 34, 66, 1, 1, 32, 64)
        memset_border(cb, 52, 34, 66, 1, 1, 32, 64)
        Rd = dram.tile([32, 64, ND * 76], f32, name="Rd")
        ident = corrpool.tile([64, 64], f32, name="ident")
        from concourse.masks import make_identity

        make_identity(nc, ident)
        l2flat = low2.rearrange("c a b -> c (a b)")
        for y in range(32):
            pR = psumR.tile([64, ND * 76], mybir.dt.float32, name="pR")
            base = y * 76
            nc.tensor.matmul(
                pR[:, 0:512],
                low1[:, 6 + y, 6:70],
                l2flat[:, base : base + 512],
                start=True,
                stop=True,
            )
            nc.tensor.matmul(
                pR[:, 512 : ND * 76],
                low1[:, 6 + y, 6:70],
                l2flat[:, base + 512 : base + ND * 76],
                start=True,
                stop=True,
            )
            rt = rpool.tile([64, ND * 76], f32, name="rt")
            act(rt, pR, AF.Identity, scale=inv_sqrt_c)
            # R row block to DRAM (contiguous per partition)
            rdst = bass.AP(
                tensor=Rd.tensor,
                offset=Rd.offset + y * 64 * ND * 76,
                ap=[[ND * 76, 64], [1, ND * 76]],
            )
            nc.sync.dma_start(out=rdst, in_=rt)
            # band gather: sh[x1, dy, dx] = R[y, x1, 76*dy + x1 + dx]
            sh = rpool.tile([64, 13, 13], f32, name="sh")
            gsrc = bass.AP(
                tensor=Rd.tensor,
                offset=Rd.offset + y * 64 * ND * 76,
                ap=[[ND * 76 + 1, 64], [76, 13], [1, 13]],
            )
            nc.sync.dma_start(out=sh, in_=gsrc)
            sh = sh.rearrange("p a b -> p (a b)")
            # transpose to channel-major via PE, copy into corr tiles
            pT = psum.tile([128, 2, 256], mybir.dt.float32, name="cp")
            nc.tensor.transpose(pT[:117, 0, 0:64], sh[:, 0:117], ident)
            act(ca[:, 1 + y, 1:65], pT[:117, 0, 0:64], AF.Identity)
            pT2 = psum.tile([128, 2, 256], mybir.dt.float32, name="cp")
            nc.tensor.transpose(pT2[:52, 0, 0:64], sh[:, 117:169], ident)
            act(cb[:, 1 + y, 1:65], pT2[:52, 0, 0:64], AF.Identity)

        # attention: att = sigmoid(W^T corr + b); corr *= att
        wa_a = wpool.tile([117, 169], f32r, name="w")
        nc.sync.dma_start(out=wa_a, in_=P["att_w"][0:117, :].bitcast(f32r))
        wa_b = wpool.tile([64, 169], f32r, name="w")
        nc.sync.dma_start(out=wa_b[:52], in_=P["att_w"][117:169, :].bitcast(f32r))
        ab_a = vecs.tile([117, 1], f32, name="att_ba")
        ab_b = vecs.tile([52, 1], f32, name="att_bb")
        src = P["att_b"][:]
        nc.sync.dma_start(
            out=ab_a,
            in_=bass.AP(tensor=src.tensor, offset=src.offset, ap=[[1, 117], [0, 1]]),
        )
        nc.sync.dma_start(
            out=ab_b,
            in_=bass.AP(tensor=src.tensor, offset=src.offset + 117, ap=[[1, 52], [0, 1]]),
        )
        att_a = corrpool.tile([117, 32, 64], f32, name="att_a")
        att_b = corrpool.tile([52, 32, 64], f32, name="att_b")
        caf = ca.rearrange("c a b -> c (a b)")
        cbf = cb.rearrange("c a b -> c (a b)")
        for mi, (m0, mb, at_, abias) in enumerate(
            [(0, 117, att_a, ab_a), (117, 52, att_b, ab_b)]
        ):
            for y0 in range(0, 32, 8):
                pa = psum.tile([128, 8, 64], mybir.dt.float32, name="cp")
                nc.tensor.matmul(
                    pa[:mb],
                    wa_a[0:117, m0 : m0 + mb],
                    ca[:, 1 + y0 : 9 + y0, 1:65],
                    start=True,
                    stop=False,
                )
                nc.tensor.matmul(
                    pa[:mb],
                    wa_b[:52, m0 : m0 + mb],
                    cb[:, 1 + y0 : 9 + y0, 1:65],
                    start=False,
                    stop=True,
                )
                act(at_[:, y0 : y0 + 8, :], pa[:mb], AF.Sigmoid, bias=abias)
        # gate in place
        nc.vector.tensor_tensor(
            out=ca[:, 1:33, 1:65], in0=ca[:, 1:33, 1:65].bitcast(f32), in1=att_a, op=ALU.mult
        )
        nc.vector.tensor_tensor(
            out=cb[:, 1:33, 1:65], in0=cb[:, 1:33, 1:65].bitcast(f32), in1=att_b, op=ALU.mult
        )

        # cc convs
        ccpool = cctx.enter_context(tc.tile_pool(name="cc", bufs=1))
        cb1p = [None, None]
        key = ("cc1_b", "pair")
        t1 = vecs.tile([128, 1], f32, name="bv_cc1_0")
        t2 = vecs.tile([128, 1], f32, name="bv_cc1_1")
        src = P["cc1_b"][:]
        nc.sync.dma_start(out=t1, in_=bass.AP(tensor=src.tensor, offset=src.offset, ap=[[1, 128], [0, 1]]))
        nc.sync.dma_start(out=t2, in_=bass.AP(tensor=src.tensor, offset=src.offset + 128, ap=[[1, 128], [0, 1]]))
        cc1a = ccpool.tile([128, 34, 66], f32r, name="cc1a")
        cc1b = ccpool.tile([128, 34, 66], f32r, name="cc1b")
        memset_border(cc1a, 128, 34, 66, 1, 1, 32, 64)
        memset_border(cc1b, 128, 34, 66, 1, 1, 32, 64)
        conv3(
            "cc1",
            [(ca, 1, 1), (cb, 1, 1)],
            [117, 52],
            "cc1_w",
            256,
            32,
            64,
            8,
            post_act([(cc1a, 1, 1), (cc1b, 1, 1)], AF.Relu, btile=[t1, t2], W=64),
        )
        ccb2 = bias_of("cc2_b", 128)
        cc2t = ccpool.tile([128, 34, 66], f32r, name="cc2t")
        memset_border(cc2t, 128, 34, 66, 1, 1, 32, 64)
        conv3(
            "cc2",
            [(cc1a, 1, 1), (cc1b, 1, 1)],
            [128, 128],
            "cc2_w",
            128,
            32,
            64,
            8,
            post_act([(cc2t, 1, 1)], AF.Relu, btile=ccb2, W=64),
        )
        ccb3 = bias_of("cc3_b", 64)
        cc3t = ccpool.tile([64, 32, 64], f32, name="cc3t")
        conv3(
            "cc3",
            [(cc2t, 1, 1)],
            [128],
            "cc3_w",
            64,
            32,
            64,
            8,
            post_act([(cc3t, 0, 0)], AF.Relu, btile=ccb3, W=64),
        )
        cfsum = vecs.tile([64, 1], f32, name="cfsum")
        with nc.allow_low_precision(reason="f32r out, fp32 bits"):
            nc.vector.tensor_reduce(cfsum, cc3t, AX.XY, ALU.add)

    # ---- heads ------------------------------------------------------------
    r1, r2 = feats[0]["rsum"], feats[1]["rsum"]
    gc1v = feats[0]["gcv"]

    def matvec(wname, kblocks, M, name):
        """kblocks: list of (vec_tile, kc). Returns psum [M,1] list per m-chunk."""
        wts = []
        k0 = 0
        for j, (v, kc) in enumerate(kblocks):
            t = wpool.tile([128, max(M, 8)], f32, name="w")
            nc.sync.dma_start(out=t[:kc, :M], in_=P[wname][k0 : k0 + kc, :])
            wts.append(t)
            k0 += kc
        outs = []
        for m0 in range(0, M, 128):
            mb = min(128, M - m0)
            pv = psum.tile([128, 2, 256], mybir.dt.float32, name="cp")
            for j, (v, kc) in enumerate(kblocks):
                nc.tensor.matmul(
                    pv[:mb, 0, 0:1],
                    wts[j][:kc, m0 : m0 + mb],
                    v[:kc],
                    start=(j == 0),
                    stop=(j == len(kblocks) - 1),
                )
            outs.append(pv)
        return outs

    # scale branch: sb1(relu) -> sb2(relu) -> sb3 -> softplus
    sb1b = bias_of("sb1_b", 128)
    pv = matvec("sb1_w", [(r1, 128), (r2, 128)], 128, "wsb1")[0]
    s1 = vecs.tile([128, 1], f32, name="s1")
    act(s1, pv[:, 0, 0:1], AF.Relu, bias=sb1b)
    sb2b = bias_of("sb2_b", 64)
    pv = matvec("sb2_w", [(s1, 128)], 64, "wsb2")[0]
    s2 = vecs.tile([64, 1], f32, name="s2")
    act(s2[:64], pv[:64, 0, 0:1], AF.Relu, bias=sb2b)
    sb3b = bias_of("sb3_b", 1)
    pv = matvec("sb3_w", [(s2, 64)], 1, "wsb3")[0]
    sc = vecs.tile([1, 1], f32, name="sc")
    act(sc[:1], pv[:1, 0, 0:1], AF.Exp, bias=sb3b)
    nc.vector.tensor_scalar_add(sc[:1], sc[:1], 1.0)
    act(sc[:1], sc[:1], AF.Ln)
    # broadcast scale to 3 partitions via DRAM bounce
    scd = dram.tile([1], f32, name="scd")
    nc.sync.dma_start(out=scd, in_=sc[0, :])
    sc3 = vecs.tile([3, 1], f32, name="sc3")
    nc.sync.dma_start(
        out=sc3, in_=bass.AP(tensor=scd.tensor, offset=scd.offset, ap=[[0, 3], [1, 1]])
    )

    # trans branch: fc1(relu) -> fc2(relu) -> fc3 -> * scale
    fb1 = [None, None, None, None]
    fc1bs = []
    src = P["fc1_b"][:]
    for j in range(4):
        t = vecs.tile([128, 1], f32, name=f"bv_fc1_{j}")
        nc.sync.dma_start(
            out=t,
            in_=bass.AP(tensor=src.tensor, offset=src.offset + 128 * j, ap=[[1, 128], [0, 1]]),
        )
        fc1bs.append(t)
    pvs = matvec("fc1_w", [(cfsum, 64), (r1, 128), (r2, 128), (gc1v, 128)], 512, "wfc1")
    t1s = []
    for j, pv in enumerate(pvs):
        tt = vecs.tile([128, 1], f32, name=f"t1_{j}")
        act(tt, pv[:, 0, 0:1], AF.Relu, bias=fc1bs[j])
        t1s.append(tt)
    fc2bs = []
    src = P["fc2_b"][:]
    for j in range(2):
        t = vecs.tile([128, 1], f32, name=f"bv_fc2_{j}")
        nc.sync.dma_start(
            out=t,
            in_=bass.AP(tensor=src.tensor, offset=src.offset + 128 * j, ap=[[1, 128], [0, 1]]),
        )
        fc2bs.append(t)
    pvs = matvec("fc2_w", [(t, 128) for t in t1s], 256, "wfc2")
    t2s = []
    for j, pv in enumerate(pvs):
        tt = vecs.tile([128, 1], f32, name=f"t2_{j}")
        act(tt, pv[:, 0, 0:1], AF.Relu, bias=fc2bs[j])
        t2s.append(tt)
    fc3b = bias_of("fc3_b", 3)
    pv = matvec("fc3_w", [(t, 128) for t in t2s], 3, "wfc3")[0]
    tr = vecs.tile([3, 1], f32, name="tr")
    act(tr[:3], pv[:3, 0, 0:1], AF.Identity, bias=fc3b)
    nc.vector.tensor_tensor(out=tr[:3], in0=tr[:3], in1=sc3[:3], op=ALU.mult)
    nc.sync.dma_start(out=out_trans[:], in_=tr[:3, 0])

    ctx.close()
    nc.compile()
    return nc


# ----------------------------------------------------------------------------
# public entry point
# ----------------------------------------------------------------------------
def kernel(img1, img2, params):
    from concourse.bass_utils import run_bass_kernel_spmd

    img1 = _np(img1)
    img2 = _np(img2)
    W = _prep_weights(params)

    if "nc" not in _CACHE:
        _CACHE["nc"] = _build()
    nc = _CACHE["nc"]

    core_ids = list(range(8))
    in_maps = []
    for i in core_ids:
        m = dict(W)
        m["img1"] = np.ascontiguousarray(img1[i])
        m["img2"] = np.ascontiguousarray(img2[i])
        in_maps.append(m)
    r = run_bass_kernel_spmd(nc, in_maps, core_ids)
    trans = np.stack([r.results[i]["trans"] for i in range(8)]).astype(np.float32)
    rot = np.zeros((8, 4), np.float32)
    rot[:, 0] = 1.0
    return rot, trans
